# revision 66
# baseline (speedup 1.0000x reference)
"""Trainium2 Bass kernel for nn_AttentiveModel (B=32,S=128,D=300,P=200,V=30000,C=3).

Data-parallel over batch across 8 NeuronCores (4 batch items per core, all
weights replicated). Trunk compute (highways/projections/compare) runs in
float32r on the PE (1 cycle/row at free>=256, near-fp32 precision); the
dist-attention elementwise runs in bf16 split across DVE and ScalarE.

Layout: activations live transposed [features(partitions), rows(free)] with
both sides sharing one 1024-col trunk (col = side*512 + b*128 + token), so
every shared-weight matmul/elementwise runs once over both sides.

att2[b,j,i] = sum_p 1/(1+|q1[b,i,p]-q2[b,j,p]|), streamed in j-blocks:
  type-A blocks (DVE abs):  w=(q1+1)-q2 (TT), v=2-w (TS 4x),
                            s=max(w,v)=1+|x| (TT), r=1/s (ScalarE Reciprocal)
  type-B blocks (ScalarE abs): d=q1-q2 (TT), |d| (ScalarE Abs),
                            r=1/(1+|d|) (ScalarE Reciprocal bias=1)
  then a DVE fold of the p=128..200 chunk onto the first 72 rows and a
  partition-sum via PE matmuls with a sliding ones-column lhsT accumulating
  into the sim PSUM tile on top of att1.
ScalarE Reciprocal is emitted directly as InstActivation (bass's wrapper
refuses it on accuracy grounds far below this problem's 2e-2 tolerance).
"""

import sys
from contextlib import ExitStack

import numpy as np

for _p in ("/opt/trn_rl_repo",):
    if _p not in sys.path:
        sys.path.insert(0, _p)

import concourse.bass as bass
import concourse.tile as tile
from concourse.bacc import Bacc
from concourse import mybir
from concourse.bass_utils import run_bass_kernel_spmd
from concourse.masks import make_identity

F32 = mybir.dt.float32
F32R = mybir.dt.float32r
BF = mybir.dt.bfloat16
H16 = mybir.dt.float16
I32 = mybir.dt.int32
ALU = mybir.AluOpType
ACTF = mybir.ActivationFunctionType
AX = mybir.AxisListType

TRUNK = F32R  # trunk compute dtype (flip to H16 to trade accuracy for SBUF)

B, S, D, P, V, C = 32, 128, 300, 200, 30000, 3
NCORES = 8
BL = B // NCORES  # 4 batch items per core
ROWS = BL * S  # 512 per side
ROWS2 = 2 * ROWS  # both sides in one trunk

CH_D = [(0, 128), (128, 128), (256, 44)]  # 300
CH_P = [(0, 128), (128, 72)]  # 200

JB = 8  # j-block size for att2 streaming (each block covers all 4 b)
NBLK = S // JB
# fraction of j-blocks whose abs runs on DVE (type A) vs ScalarE (type B)
A_NUM, A_DEN = 1, 2

WEIGHT_NAMES = [
    "hw1_Wh", "hw1_bh", "hw1_Wt", "hw1_bt",
    "hw2_Wh", "hw2_bh", "hw2_Wt", "hw2_bt",
    "mul_W1", "mul_b1", "mul_W2", "mul_b2",
    "dist_W1", "dist_b1", "dist_W2", "dist_b2",
    "cmp_W1", "cmp_b1", "cmp_W2", "cmp_b2",
    "chw1_Wh", "chw1_bh", "chw1_Wt", "chw1_bt",
    "chw2_Wh", "chw2_bh", "chw2_Wt", "chw2_bt",
    "agg_W1", "agg_b1", "agg_W2", "agg_b2",
    "out_W", "out_b",
]

# weights kept fp32 (tiny free dims in the aggregate MLP)
F32_WEIGHTS = {"agg_W1", "agg_W2", "out_W"}


def _chunks(n):
    out = []
    o = 0
    while o < n:
        c = min(128, n - o)
        out.append((o, c))
        o += c
    return out


def act_recip(nc, out, in_, bias=0.0):
    """out = 1/(in_ + bias) in one ScalarE pass (Reciprocal activation)."""
    eng = nc.scalar
    ins_ = [
        eng.lower_ap(in_),
        mybir.ImmediateValue(dtype=mybir.dt.float32, value=bias),  # bias
        mybir.ImmediateValue(dtype=mybir.dt.float32, value=1.0),  # scale
        mybir.ImmediateValue(dtype=mybir.dt.float32, value=0.0),  # alpha
    ]
    return eng.add_instruction(
        mybir.InstActivation(
            name=eng.bass.get_next_instruction_name(),
            func=ACTF.Reciprocal,
            ins=ins_,
            outs=[eng.lower_ap(out)],
        )
    )


def build_nc(debug=False):
    nc = Bacc()

    io = {}
    io["x1"] = nc.declare_dram_parameter("x1", [BL, S], I32, isOutput=False)
    io["x2"] = nc.declare_dram_parameter("x2", [BL, S], I32, isOutput=False)
    io["emb"] = nc.declare_dram_parameter("emb", [V, D], F32, isOutput=False)
    shapes = {
        "hw1_Wh": [D, D], "hw1_bh": [D], "hw1_Wt": [D, D], "hw1_bt": [D],
        "hw2_Wh": [D, D], "hw2_bh": [D], "hw2_Wt": [D, D], "hw2_bt": [D],
        "mul_W1": [D, P], "mul_b1": [P], "mul_W2": [P, P], "mul_b2": [P],
        "dist_W1": [D, P], "dist_b1": [P], "dist_W2": [P, P], "dist_b2": [P],
        "cmp_W1": [4 * D, P], "cmp_b1": [P], "cmp_W2": [P, P], "cmp_b2": [P],
        "chw1_Wh": [P, P], "chw1_bh": [P], "chw1_Wt": [P, P], "chw1_bt": [P],
        "chw2_Wh": [P, P], "chw2_bh": [P], "chw2_Wt": [P, P], "chw2_bt": [P],
        "agg_W1": [4 * P, P], "agg_b1": [P], "agg_W2": [P, P], "agg_b2": [P],
        "out_W": [P, C], "out_b": [C],
    }
    for n in WEIGHT_NAMES:
        io[n] = nc.declare_dram_parameter(n, shapes[n], F32, isOutput=False)
    io["yt"] = nc.declare_dram_parameter("yt", [C, BL], F32, isOutput=True)
    if debug:
        io["dbg_eTh0"] = nc.declare_dram_parameter("dbg_eTh0", [128, ROWS2], F32, isOutput=True)
        io["dbg_qT0"] = nc.declare_dram_parameter("dbg_qT0", [128, ROWS2], F32, isOutput=True)
        io["dbg_sim4"] = nc.declare_dram_parameter("dbg_sim4", [128, 512], F32, isOutput=True)
        io["dbg_betaT0"] = nc.declare_dram_parameter("dbg_betaT0", [128, 512], F32, isOutput=True)
        io["dbg_vT0"] = nc.declare_dram_parameter("dbg_vT0", [128, ROWS2], F32, isOutput=True)

    with ExitStack() as ctx:
        tc = ctx.enter_context(tile.TileContext(nc))
        _emit(ctx, nc, tc, io, debug=debug)
    nc.finalize()
    return nc


def _emit(ctx, nc, tc, io, debug=False):
    def dbg_dump(name, ap):
        if not debug or name not in io:
            return
        sh = io[name].shape
        src = ap[:sh[0], :sh[1]]
        if src.space == bass.MemorySpace.PSUM:
            t = small.tile([128, 512], F32, tag="dbgps", name=name)
            nc.scalar.activation(out=t[:sh[0], :sh[1]], in_=src, func=ACTF.Copy)
            src = t[:sh[0], :sh[1]]
        nc.gpsimd.dma_start(out=io[name][:, :], in_=src)

    wpool = ctx.enter_context(tc.tile_pool(name="wpool", bufs=1))
    const = ctx.enter_context(tc.tile_pool(name="const", bufs=1))
    persist = ctx.enter_context(tc.tile_pool(name="persist", bufs=1))
    work = ctx.enter_context(tc.tile_pool(name="work", bufs=1))
    # u triple-buffers on hardware; drop to 2 in debug builds to make room
    # for the debug dump staging (timing is irrelevant in CoreSim)
    upool = ctx.enter_context(tc.tile_pool(name="upool", bufs=(2 if debug else 3)))
    vpool = ctx.enter_context(tc.tile_pool(name="vpool", bufs=1))
    small = ctx.enter_context(tc.tile_pool(name="small", bufs=2))

    pp_mm = ctx.enter_context(tc.tile_pool(name="pp_mm", bufs=2, space="PSUM"))
    pp_sim = ctx.enter_context(tc.tile_pool(name="pp_sim", bufs=1, space="PSUM"))
    pp_tr = ctx.enter_context(tc.tile_pool(name="pp_tr", bufs=2, space="PSUM"))
    pp_r = ctx.enter_context(tc.tile_pool(name="pp_r", bufs=1, space="PSUM"))
    pp_sm = ctx.enter_context(tc.tile_pool(name="pp_sm", bufs=2, space="PSUM"))

    # ---------------- constants ----------------
    identf = const.tile([128, 128], F32, tag="identf", name="identf")
    make_identity(nc, identf[:, :])
    identr = const.tile([128, 128], TRUNK, tag="identr", name="identr")
    nc.vector.tensor_scalar_add(out=identr[:, :], in0=identf[:, :], scalar1=0.0)
    identb = const.tile([128, 128], H16, tag="identb", name="identb")
    nc.vector.tensor_scalar_add(out=identb[:, :], in0=identf[:, :], scalar1=0.0)

    # sliding ones-column buffer: Z[:, 32] == 1 so Z[:, 32-r:64-r] has its
    # ones in column r; Z_slice.T @ U deposits column-sums of U into row r.
    zbuf = const.tile([128, 64], H16, tag="zbuf", name="zbuf")
    nc.vector.memset(zbuf[:, :], 0.0)
    nc.vector.memset(zbuf[:, 32:33], 1.0)

    neg1 = const.tile([128, 1], F32, tag="neg1", name="neg1")
    nc.vector.memset(neg1[:, :], -1.0)

    # ---------------- weights: casting DMAs via gpsimd queue --------------
    SPECIAL_KCH = {
        "cmp_W1": [(s * D + o, c) for s in range(4) for (o, c) in CH_D],
        "agg_W1": [(s * P + o, c) for s in range(4) for (o, c) in CH_P],
    }

    def load_w(name):
        h = io[name]
        K, M = h.shape
        H16_W = {"cmp_W1", "cmp_W2", "chw1_Wh", "chw1_Wt", "chw2_Wh", "chw2_Wt"}
        dt = F32 if name in F32_WEIGHTS else (H16 if name in H16_W else TRUNK)
        tiles = []
        for i, (o, c) in enumerate(SPECIAL_KCH.get(name, _chunks(K))):
            t = wpool.tile([c, M], dt, tag=f"w_{name}_{i}", name=f"w_{name}_{i}")
            eng = nc.sync if dt == F32 else nc.gpsimd
            eng.dma_start(out=t[:, :], in_=h[o:o + c, :])
            tiles.append(t)
        return tiles

    def load_b(name):
        h = io[name]
        (M,) = h.shape
        tiles = []
        for i, (o, c) in enumerate(_chunks(M)):
            t = wpool.tile([c, 1], F32, tag=f"b_{name}_{i}", name=f"b_{name}_{i}")
            nc.sync.dma_start(out=t[:, :], in_=h[o:o + c])
            tiles.append(t)
        return tiles

    # ---------------- index DMAs + gathers (overlap weight DMAs) ----------
    pre2 = ctx.enter_context(ExitStack())
    g2pool = pre2.enter_context(tc.tile_pool(name="g2pool", bufs=1))
    pre1 = ctx.enter_context(ExitStack())
    gpool = pre1.enter_context(tc.tile_pool(name="gpool", bufs=1))
    e_n = {}
    for side, xh in (("1", io["x1"]), ("2", io["x2"])):
        for b in range(BL):
            idx = gpool.tile([128, 1], I32, tag=f"idx{side}_{b}", name=f"idx{side}_{b}")
            nc.sync.dma_start(out=idx[:, :], in_=xh[b, :])
            e = gpool.tile([128, D], H16, tag=f"e{side}_{b}", name=f"e{side}_{b}")
            nc.gpsimd.indirect_dma_start(
                out=e[:, :], out_offset=None, in_=io["emb"][:, :],
                in_offset=bass.IndirectOffsetOnAxis(ap=idx[:, :1], axis=0),
            )
            e_n[(side, b)] = e

    W = {}
    for n in WEIGHT_NAMES:
        W[n] = load_b(n) if n.endswith(("bh", "bt", "b1", "b2", "_b")) else load_w(n)

    # ---------------- helpers ----------------
    def mm_apply(w_tiles, b_tiles, rhs_tiles, n_free, func, out_tiles,
                 krange=None, mrange=None):
        """out = func(W.T @ rhs + b), transposed layout, 512-col PSUM chunks."""
        M = w_tiles[0].shape[1]
        mch = _chunks(M)
        ks = list(range(len(w_tiles))) if krange is None else krange
        m_iter = ([(i, i) for i in range(len(mch))] if mrange is None
                  else list(enumerate(mrange)))
        for oi, mi in m_iter:
            mo, mc = mch[mi]
            for fo in range(0, n_free, 512):
                fc = min(512, n_free - fo)
                ps = pp_mm.tile([128, 512], F32, tag="mmout", name="mmout")
                for idx, ki in enumerate(ks):
                    kc = w_tiles[ki].shape[0]
                    nc.tensor.matmul(
                        out=ps[:mc, :fc],
                        lhsT=w_tiles[ki][:kc, mo:mo + mc],
                        rhs=rhs_tiles[ki][:kc, fo:fo + fc],
                        start=(idx == 0),
                        stop=(idx == len(ks) - 1),
                    )
                nc.scalar.activation(
                    out=out_tiles[oi][:mc, fo:fo + fc],
                    in_=ps[:mc, :fc],
                    func=func, bias=b_tiles[mi][:mc, :], scale=1.0,
                )

    def highway(xt_tiles, wh, bh, wt, bt, feat, out_tiles):
        """out = x + t*(h-x), trunk layout, chunk-at-a-time (h reused as tmp)."""
        ch = _chunks(feat)
        for mi, (mo, mc) in enumerate(ch):
            h = work.tile([128, ROWS2], TRUNK, tag="hw_h", name="hw_h")
            t = work.tile([128, ROWS2], TRUNK, tag="hw_t", name="hw_t")
            mm_apply(wh, bh, xt_tiles, ROWS2, ACTF.Relu, [h], mrange=[mi])
            mm_apply(wt, bt, xt_tiles, ROWS2, ACTF.Sigmoid, [t], mrange=[mi])
            x_sl = xt_tiles[mi][:mc, :]
            nc.vector.tensor_tensor(out=h[:mc, :], in0=h[:mc, :], in1=x_sl,
                                    op=ALU.subtract)
            nc.vector.tensor_tensor(out=h[:mc, :], in0=h[:mc, :], in1=t[:mc, :],
                                    op=ALU.mult)
            nc.vector.tensor_tensor(out=out_tiles[mi][:mc, :], in0=h[:mc, :],
                                    in1=x_sl, op=ALU.add)

    # ---------------- embed: transpose into trunk ----------------
    # eT[ki]: [kc, 1024], col = side*512 + b*128 + token
    eT = [g2pool.tile([128, ROWS2], TRUNK, tag=f"eT_{i}", name=f"eT_{i}")
          for i in range(3)]
    for ki, (ko, kc) in enumerate(CH_D):
        for side in ("1", "2"):
            ps = pp_tr.tile([128, 512], H16, tag="trpackb", name="trpack")
            for b in range(BL):
                nc.tensor.transpose(
                    out=ps[:kc, b * S:(b + 1) * S],
                    in_=e_n[(side, b)][:, ko:ko + kc],
                    identity=identb[:128, :128],
                )
            so = (0 if side == "1" else ROWS)
            nc.scalar.activation(out=eT[ki][:kc, so:so + ROWS], in_=ps[:kc, :ROWS],
                                 func=ACTF.Copy)
    pre1.close()  # frees index + gather tiles

    # ---------------- highway stack (trunk: both sides at once) -------------
    h1 = [g2pool.tile([128, ROWS2], TRUNK, tag=f"hwy1_{i}", name=f"hwy1_{i}")
          for i in range(3)]
    highway(eT, W["hw1_Wh"], W["hw1_bh"], W["hw1_Wt"], W["hw1_bt"], D, h1)
    eTh = [persist.tile([128, ROWS2], TRUNK, tag=f"eTh_{i}", name=f"eTh_{i}")
           for i in range(3)]
    highway(h1, W["hw2_Wh"], W["hw2_bh"], W["hw2_Wt"], W["hw2_bt"], D, eTh)
    pre2.close()  # frees eT, h1

    # normal-layout post-highway embeddings (lhsT for the beta/alpha matmuls)
    ehw_n = {}
    for side in ("1", "2"):
        so = (0 if side == "1" else ROWS)
        for b in range(BL):
            ps = pp_r.tile([128, 512], TRUNK, tag="trpackr", name="trpackr")
            for ki, (ko, kc) in enumerate(CH_D):
                nc.tensor.transpose(
                    out=ps[:128, ko:ko + kc],
                    in_=eTh[ki][:kc, so + b * S:so + (b + 1) * S],
                    identity=identr[:kc, :kc],
                )
            t = persist.tile([128, D], H16, tag=f"ehwn{side}_{b}", name=f"ehwn{side}_{b}")
            nc.scalar.activation(out=t[:, :], in_=ps[:, :D], func=ACTF.Copy)
            ehw_n[(side, b)] = t

    # ---------------- projections (shared weights, trunk) ----------------
    def proj(prefix, pool):
        z1 = [work.tile([128, ROWS2], TRUNK, tag=f"z1_{i}", name=f"z1_{i}") for i in range(2)]
        mm_apply(W[f"{prefix}_W1"], W[f"{prefix}_b1"], eTh, ROWS2, ACTF.Relu, z1)
        out = [pool.tile([128, ROWS2], TRUNK, tag=f"{prefix}T_{i}", name=f"{prefix}T_{i}")
               for i in range(2)]
        mm_apply(W[f"{prefix}_W2"], W[f"{prefix}_b2"], z1, ROWS2, ACTF.Relu, out)
        return out

    # dist first so the att2 elementwise can start while the PE continues
    # with the mul projection (att1 is only needed at b0's first fold_sum)
    qscope = ctx.enter_context(ExitStack())
    qpool = qscope.enter_context(tc.tile_pool(name="qpool", bufs=1))
    qT = proj("dist", qpool)
    dbg_dump("dbg_eTh0", eTh[0][:, :])
    dbg_dump("dbg_qT0", qT[0][:, :])

    # fp16 views of q for the att2 elementwise: q1p = q1+1, q2b = q2.
    # The unused partition rows 72:128 of the low chunk are zeroed so the
    # subtract can run over all 128 partitions without stale data.
    q1p, q2b = [], []
    for ki in range(2):
        kc = CH_P[ki][1]
        tp = persist.tile([128, ROWS], H16, tag=f"q1p_{ki}", name=f"q1p_{ki}")
        t2 = persist.tile([128, ROWS], H16, tag=f"q2b_{ki}", name=f"q2b_{ki}")
        if kc < 128:
            nc.vector.memset(tp[64:128, :], 0.0)
            nc.vector.memset(t2[64:128, :], 0.0)
        nc.vector.tensor_scalar_add(out=tp[:kc, :], in0=qT[ki][:kc, :ROWS], scalar1=1.0)
        nc.vector.tensor_scalar_add(out=t2[:kc, :], in0=qT[ki][:kc, ROWS:], scalar1=0.0)
        q1p.append(tp)
        q2b.append(t2)
    qscope.close()  # frees qT

    pT = proj("mul", persist)

    # ---------------- att1 into sim4 PSUM (simT layout [j, i] per b) --------
    # start=True lazily marks the WHOLE bank pending-zero, so only the very
    # first matmul touching sim4 carries it; every later first-touch of a byte
    # overwrites, and overlapping writes accumulate. The att2 sums then
    # accumulate on top with start=False.
    sim4 = pp_sim.tile([128, 512], F32, tag="sim4", name="sim4")
    for b in range(BL):
        for ki, (ko, kc) in enumerate(CH_P):
            nc.tensor.matmul(
                out=sim4[:, b * S:(b + 1) * S],
                lhsT=pT[ki][:kc, ROWS + b * S:ROWS + (b + 1) * S],
                rhs=pT[ki][:kc, b * S:(b + 1) * S],
                start=(b == 0 and ki == 0), stop=False, skip_group_check=True,
            )

    # ---------------- att2: dist attention ----------------
    # Each j-block covers ALL 4 batch items: u layout [p, (j, b, i)] so the
    # partition-sum matmuls take 512-wide rhs slices (one matmul per j) and
    # write all of sim4's row [32g+rr, :].
    half = JB * 512  # hi/lo chunk size in u (j, b, i)

    def att2_block(jb, type_a):
        u = upool.tile([128, 2 * half], H16, tag="u", name="u")
        for ki, off in ((0, 0), (1, half)):
            uo = u[:128, off:off + half].rearrange(
                "p (j b i) -> p j b i", j=JB, b=BL)
            q1s = q1p[ki][:128, :]
            in0 = bass.AP(tensor=q1s.tensor, offset=q1s.offset,
                          ap=[q1s.ap[0], [0, JB], [S, BL], [1, S]])
            q2s = q2b[ki][:128, jb * JB:jb * JB + JB]
            in1 = bass.AP(tensor=q2s.tensor, offset=q2s.offset,
                          ap=[q2s.ap[0], q2s.ap[1], [S, BL], [0, S]])
            nc.vector.tensor_tensor(out=uo, in0=in0, in1=in1, op=ALU.subtract)
        if type_a:
            # v = 2-u = 1-x ; u = max(u,v) = 1+|x| ; u = 1/u
            v = vpool.tile([128, 2 * half], H16, tag="v", name="v")
            nc.vector.tensor_scalar(out=v[:, :], in0=u[:, :], scalar1=-1.0,
                                    scalar2=2.0, op0=ALU.mult, op1=ALU.add)
            nc.vector.tensor_tensor(out=u[:, :], in0=u[:, :], in1=v[:, :],
                                    op=ALU.max)
            act_recip(nc, u[:, :], u[:, :], bias=0.0)
        else:
            # u = |u-1| = |x| ; u = 1/(1+u)
            nc.scalar.activation(out=u[:, :], in_=u[:, :], func=ACTF.Abs,
                                 bias=neg1[:, :])
            act_recip(nc, u[:, :], u[:, :], bias=1.0)
        return u

    def att2_fold_sum(jb, u):
        # fold the p=128..200 chunk onto the first 72 rows of the hi chunk on
        # the otherwise-idle gpsimd: it runs concurrently with the NEXT
        # block's DVE work instead of queueing behind it
        nc.gpsimd.tensor_tensor(out=u[:72, :half], in0=u[:72, :half],
                                in1=u[:72, half:], op=ALU.add)
        for jj in range(JB):
            j = jb * JB + jj
            g, rr = j // 32, j % 32
            nc.tensor.matmul(
                out=sim4[32 * g:32 * g + 32, :],
                lhsT=zbuf[:128, 32 - rr:64 - rr],
                rhs=u[:128, jj * 512:(jj + 1) * 512],
                start=False, stop=(rr == 31), skip_group_check=True,
                tile_position=(0, 32 * g),
            )

    prev = None
    for jb in range(NBLK):
        type_a = (jb * A_NUM) % A_DEN < A_NUM
        u = att2_block(jb, type_a)
        if prev is not None:
            att2_fold_sum(*prev)
        prev = (jb, u)
    att2_fold_sum(*prev)
    dbg_dump("dbg_sim4", sim4[:, :])

    # ---------------- softmax + beta/alpha + compare part 1 ----------------
    def softmax_p(src_psum):
        """softmax over rows of src [128,128]; returns transposed probs bf16."""
        mx = small.tile([128, 1], F32, tag="sm_mx", name="sm_mx")
        nc.vector.tensor_reduce(out=mx[:, :], in_=src_psum, axis=AX.X,
                                op=ALU.max, negate=True)
        esb = small.tile([128, S], H16, tag="sm_e", name="sm_e")
        zs = small.tile([128, 1], F32, tag="sm_z", name="sm_z")
        nc.scalar.activation(out=esb[:, :], in_=src_psum, func=ACTF.Exp,
                             bias=mx[:, :], scale=1.0, accum_out=zs[:, :])
        rz = small.tile([128, 1], F32, tag="sm_rz", name="sm_rz")
        nc.vector.reciprocal(out=rz[:, :], in_=zs[:, :])
        pr = small.tile([128, S], H16, tag="sm_p", name="sm_p")
        nc.vector.tensor_scalar(out=pr[:, :], in0=esb[:, :], scalar1=rz[:, :],
                                scalar2=None, op0=ALU.mult)
        pt_ps = pp_tr.tile([128, 512], H16, tag="trpackb", name="trpackb")
        nc.tensor.transpose(out=pt_ps[:S, :S], in_=pr[:, :], identity=identb[:, :])
        pt = small.tile([128, S], H16, tag="sm_pt", name="sm_pt")
        nc.scalar.activation(out=pt[:, :], in_=pt_ps[:S, :S], func=ACTF.Copy)
        return pt

    # betaT trunk tiles per side: [kc, 512] bf16, col = b*128 + token
    betaT = {s: [persist.tile([128, 512], H16, tag=f"betaT{s}_{i}", name=f"betaT{s}_{i}")
                 for i in range(3)] for s in ("1", "2")}

    for b in range(BL):
        bs4 = sim4[:, b * S:(b + 1) * S]
        ptA = softmax_p(bs4)  # alpha probs^T [i, j]
        simT_sb = small.tile([128, S], F32, tag="simT_sb", name="simT_sb")
        nc.scalar.activation(out=simT_sb[:, :], in_=bs4, func=ACTF.Copy)
        sim_ps = pp_sm.tile([128, S], F32, tag="btps", name="simtr")
        nc.tensor.transpose(out=sim_ps[:S, :S], in_=simT_sb[:, :],
                            identity=identf[:, :])
        ptB = softmax_p(sim_ps[:S, :S])  # beta probs^T [j, i]

        for side, pt, eln in (("1", ptB, "2"), ("2", ptA, "1")):
            for ki, (ko, kc) in enumerate(CH_D):
                bt_ps = pp_sm.tile([128, S], F32, tag="btps", name="btps")
                nc.tensor.matmul(
                    out=bt_ps[:kc, :], lhsT=ehw_n[(eln, b)][:, ko:ko + kc],
                    rhs=pt[:, :], start=True, stop=True,
                )
                nc.scalar.activation(
                    out=betaT[side][ki][:kc, b * S:(b + 1) * S],
                    in_=bt_ps[:kc, :], func=ACTF.Copy)

    dbg_dump("dbg_betaT0", betaT["1"][0][:, :])

    # cat + compare matmul, per side over 512-col trunk halves. The cat
    # chunks (e-b, e*b) are computed on the fly right before their two
    # accumulating matmuls, so only 2 transient tiles are alive at a time.
    cmp1 = [persist.tile([128, ROWS2], H16, tag=f"cmp1_{i}", name=f"cmp1_{i}")
            for i in range(2)]
    for side in ("1", "2"):
        so = (0 if side == "1" else ROWS)
        ps2 = [pp_mm.tile([128, 512], F32, tag="mmout", name=f"cmp1ps{mi}")
               for mi in range(2)]
        for sel in range(4):  # e, beta, e-beta, e*beta
            for ki, (ko, kc) in enumerate(CH_D):
                e_sl = eTh[ki][:kc, so:so + ROWS]
                b_sl = betaT[side][ki][:kc, :]
                if sel == 1:
                    rhs = b_sl
                else:
                    cat = small.tile([128, 512], H16, tag="cat", name="cat")
                    if sel == 0:
                        nc.vector.tensor_scalar_add(out=cat[:kc, :], in0=e_sl,
                                                    scalar1=0.0)
                    else:
                        nc.vector.tensor_tensor(
                            out=cat[:kc, :], in0=e_sl, in1=b_sl,
                            op=(ALU.subtract if sel == 2 else ALU.mult))
                    rhs = cat[:kc, :]
                idx = sel * 3 + ki
                for mi, (mo, mc) in enumerate(CH_P):
                    nc.tensor.matmul(
                        out=ps2[mi][:mc, :],
                        lhsT=W["cmp_W1"][idx][:, mo:mo + mc],
                        rhs=rhs,
                        start=(idx == 0), stop=(idx == 11),
                        skip_group_check=True,
                    )
        for mi, (mo, mc) in enumerate(CH_P):
            nc.scalar.activation(
                out=cmp1[mi][:mc, so:so + ROWS], in_=ps2[mi][:mc, :],
                func=ACTF.Relu,
                bias=W["cmp_b1"][mi][:mc, :], scale=1.0,
            )

    # ---------------- compare part 2 + compare highway (trunk) --------------
    v0 = [work.tile([128, ROWS2], H16, tag=f"v0_{i}", name=f"v0_{i}") for i in range(2)]
    mm_apply(W["cmp_W2"], W["cmp_b2"], cmp1, ROWS2, ACTF.Relu, v0)
    v1 = [work.tile([128, ROWS2], H16, tag=f"v1_{i}", name=f"v1_{i}") for i in range(2)]
    highway(v0, W["chw1_Wh"], W["chw1_bh"], W["chw1_Wt"], W["chw1_bt"], P, v1)
    vT = [persist.tile([128, ROWS2], H16, tag=f"vT_{i}", name=f"vT_{i}")
          for i in range(2)]
    highway(v1, W["chw2_Wh"], W["chw2_bh"], W["chw2_Wt"], W["chw2_bt"], P, vT)
    dbg_dump("dbg_vT0", vT[0][:, :])

    # ---------------- aggregate (fp32) ----------------
    # stats[sect][ki]: [kc, BL]; sections: v1.max, v2.max, v1.sum, v2.sum
    stats = []
    for sect, (side, op) in enumerate(
            (("1", ALU.max), ("2", ALU.max), ("1", ALU.add), ("2", ALU.add))):
        so = (0 if side == "1" else ROWS)
        st = [persist.tile([128, BL], F32, tag=f"st{sect}_{i}", name=f"st{sect}_{i}")
              for i in range(2)]
        for ki, (ko, kc) in enumerate(CH_P):
            seg = vT[ki][:kc, so:so + ROWS].rearrange("p (b t) -> p b t", b=BL)
            nc.vector.tensor_reduce(
                out=st[ki][:kc, :BL], in_=seg, axis=AX.X, op=op,
            )
        stats.append(st)

    agg_rhs = [stats[s][ki] for s in range(4) for ki in range(2)]
    y1 = [persist.tile([128, BL], F32, tag=f"y1_{i}", name=f"y1_{i}") for i in range(2)]
    mm_apply(W["agg_W1"], W["agg_b1"], agg_rhs, BL, ACTF.Relu, y1)
    y2 = [persist.tile([128, BL], F32, tag=f"y2_{i}", name=f"y2_{i}") for i in range(2)]
    mm_apply(W["agg_W2"], W["agg_b2"], y1, BL, ACTF.Relu, y2)

    yt_ps = pp_sm.tile([128, S], F32, tag="btps", name="btps")
    for ki, (ko, kc) in enumerate(CH_P):
        nc.tensor.matmul(
            out=yt_ps[:C, :BL], lhsT=W["out_W"][ki][:kc, :],
            rhs=y2[ki][:kc, :], start=(ki == 0), stop=(ki == 1),
        )
    yt_sb = persist.tile([C, BL], F32, tag="yt_sb", name="yt_sb")
    nc.scalar.activation(out=yt_sb[:, :], in_=yt_ps[:C, :BL], func=ACTF.Identity,
                         bias=W["out_b"][0][:C, :], scale=1.0)
    nc.sync.dma_start(out=io["yt"][:, :], in_=yt_sb[:, :])


_NC_CACHE = {}


def _get_nc():
    if "nc" not in _NC_CACHE:
        _NC_CACHE["nc"] = build_nc()
    return _NC_CACHE["nc"]


def make_in_maps(inputs):
    """Shard full inputs into 8 per-core input maps."""
    x1 = np.ascontiguousarray(np.asarray(inputs["x1"]).astype(np.int32))
    x2 = np.ascontiguousarray(np.asarray(inputs["x2"]).astype(np.int32))
    shared = {}
    for n in WEIGHT_NAMES + ["emb"]:
        shared[n] = np.ascontiguousarray(np.asarray(inputs[n]).astype(np.float32))
    in_maps = []
    for c in range(NCORES):
        m = dict(shared)
        m["x1"] = x1[c * BL:(c + 1) * BL]
        m["x2"] = x2[c * BL:(c + 1) * BL]
        in_maps.append(m)
    return in_maps


def kernel(**inputs):
    nc = _get_nc()
    in_maps = make_in_maps(inputs)
    res = run_bass_kernel_spmd(nc, in_maps, core_ids=list(range(NCORES)))
    return np.concatenate([np.asarray(r["yt"]).T for r in res.results], axis=0)


if __name__ == "__main__":
    nc = build_nc()
    print("built ok")


# revision 69
# speedup vs baseline: 1.2677x; 1.2677x over previous
"""Trainium2 Bass kernel for nn_AttentiveModel (B=32,S=128,D=300,P=200,V=30000,C=3).

Data-parallel over batch across 8 NeuronCores (4 batch items per core, all
weights replicated). Trunk compute (highways/projections/compare) runs in
float32r on the PE (1 cycle/row at free>=256, near-fp32 precision); the
dist-attention elementwise runs in bf16 split across DVE and ScalarE.

Layout: activations live transposed [features(partitions), rows(free)] with
both sides sharing one 1024-col trunk (col = side*512 + b*128 + token), so
every shared-weight matmul/elementwise runs once over both sides.

att2[b,j,i] = sum_p 1/(1+|q1[b,i,p]-q2[b,j,p]|), streamed in j-blocks:
  type-A blocks (DVE abs):  w=(q1+1)-q2 (TT), v=2-w (TS 4x),
                            s=max(w,v)=1+|x| (TT), r=1/s (ScalarE Reciprocal)
  type-B blocks (ScalarE abs): d=q1-q2 (TT), |d| (ScalarE Abs),
                            r=1/(1+|d|) (ScalarE Reciprocal bias=1)
  then a DVE fold of the p=128..200 chunk onto the first 72 rows and a
  partition-sum via PE matmuls with a sliding ones-column lhsT accumulating
  into the sim PSUM tile on top of att1.
ScalarE Reciprocal is emitted directly as InstActivation (bass's wrapper
refuses it on accuracy grounds far below this problem's 2e-2 tolerance).
"""

import sys
from contextlib import ExitStack

import numpy as np

for _p in ("/opt/trn_rl_repo",):
    if _p not in sys.path:
        sys.path.insert(0, _p)

import concourse.bass as bass
import concourse.tile as tile
from concourse.bacc import Bacc
from concourse import mybir
from concourse.bass_utils import run_bass_kernel_spmd
from concourse.masks import make_identity

F32 = mybir.dt.float32
F32R = mybir.dt.float32r
BF = mybir.dt.bfloat16
H16 = mybir.dt.float16
I32 = mybir.dt.int32
ALU = mybir.AluOpType
ACTF = mybir.ActivationFunctionType
AX = mybir.AxisListType

TRUNK = F32R  # trunk compute dtype (flip to H16 to trade accuracy for SBUF)

B, S, D, P, V, C = 32, 128, 300, 200, 30000, 3
NCORES = 8
BL = B // NCORES  # 4 batch items per core
ROWS = BL * S  # 512 per side
ROWS2 = 2 * ROWS  # both sides in one trunk

CH_D = [(0, 128), (128, 128), (256, 44)]  # 300
CH_P = [(0, 128), (128, 72)]  # 200

JB = 8  # j-block size for att2 streaming (each block covers all 4 b)
NBLK = S // JB
# fraction of j-blocks whose abs runs on DVE (type A) vs ScalarE (type B)
A_NUM, A_DEN = 1, 2

WEIGHT_NAMES = [
    "hw1_Wh", "hw1_bh", "hw1_Wt", "hw1_bt",
    "hw2_Wh", "hw2_bh", "hw2_Wt", "hw2_bt",
    "mul_W1", "mul_b1", "mul_W2", "mul_b2",
    "dist_W1", "dist_b1", "dist_W2", "dist_b2",
    "cmp_W1", "cmp_b1", "cmp_W2", "cmp_b2",
    "chw1_Wh", "chw1_bh", "chw1_Wt", "chw1_bt",
    "chw2_Wh", "chw2_bh", "chw2_Wt", "chw2_bt",
    "agg_W1", "agg_b1", "agg_W2", "agg_b2",
    "out_W", "out_b",
]

# weights kept fp32 (tiny free dims in the aggregate MLP)
F32_WEIGHTS = {"agg_W1", "agg_W2", "out_W"}


def _chunks(n):
    out = []
    o = 0
    while o < n:
        c = min(128, n - o)
        out.append((o, c))
        o += c
    return out


def act_recip(nc, out, in_, bias=0.0):
    """out = 1/(in_ + bias) in one ScalarE pass (Reciprocal activation)."""
    eng = nc.scalar
    ins_ = [
        eng.lower_ap(in_),
        mybir.ImmediateValue(dtype=mybir.dt.float32, value=bias),  # bias
        mybir.ImmediateValue(dtype=mybir.dt.float32, value=1.0),  # scale
        mybir.ImmediateValue(dtype=mybir.dt.float32, value=0.0),  # alpha
    ]
    return eng.add_instruction(
        mybir.InstActivation(
            name=eng.bass.get_next_instruction_name(),
            func=ACTF.Reciprocal,
            ins=ins_,
            outs=[eng.lower_ap(out)],
        )
    )


def build_nc(debug=False):
    nc = Bacc()

    io = {}
    io["x1"] = nc.declare_dram_parameter("x1", [BL, S], I32, isOutput=False)
    io["x2"] = nc.declare_dram_parameter("x2", [BL, S], I32, isOutput=False)
    io["emb"] = nc.declare_dram_parameter("emb", [V, D], F32, isOutput=False)
    shapes = {
        "hw1_Wh": [D, D], "hw1_bh": [D], "hw1_Wt": [D, D], "hw1_bt": [D],
        "hw2_Wh": [D, D], "hw2_bh": [D], "hw2_Wt": [D, D], "hw2_bt": [D],
        "mul_W1": [D, P], "mul_b1": [P], "mul_W2": [P, P], "mul_b2": [P],
        "dist_W1": [D, P], "dist_b1": [P], "dist_W2": [P, P], "dist_b2": [P],
        "cmp_W1": [4 * D, P], "cmp_b1": [P], "cmp_W2": [P, P], "cmp_b2": [P],
        "chw1_Wh": [P, P], "chw1_bh": [P], "chw1_Wt": [P, P], "chw1_bt": [P],
        "chw2_Wh": [P, P], "chw2_bh": [P], "chw2_Wt": [P, P], "chw2_bt": [P],
        "agg_W1": [4 * P, P], "agg_b1": [P], "agg_W2": [P, P], "agg_b2": [P],
        "out_W": [P, C], "out_b": [C],
    }
    for n in WEIGHT_NAMES:
        io[n] = nc.declare_dram_parameter(n, shapes[n], F32, isOutput=False)
    io["yt"] = nc.declare_dram_parameter("yt", [C, BL], F32, isOutput=True)
    if debug:
        io["dbg_eTh0"] = nc.declare_dram_parameter("dbg_eTh0", [128, ROWS2], F32, isOutput=True)
        io["dbg_qT0"] = nc.declare_dram_parameter("dbg_qT0", [128, ROWS2], F32, isOutput=True)
        io["dbg_sim4"] = nc.declare_dram_parameter("dbg_sim4", [128, 512], F32, isOutput=True)
        io["dbg_betaT0"] = nc.declare_dram_parameter("dbg_betaT0", [128, 512], F32, isOutput=True)
        io["dbg_vT0"] = nc.declare_dram_parameter("dbg_vT0", [128, ROWS2], F32, isOutput=True)

    with ExitStack() as ctx:
        tc = ctx.enter_context(tile.TileContext(nc))
        _emit(ctx, nc, tc, io, debug=debug)
    nc.finalize()
    return nc


def _emit(ctx, nc, tc, io, debug=False):
    def dbg_dump(name, ap):
        if not debug or name not in io:
            return
        sh = io[name].shape
        src = ap[:sh[0], :sh[1]]
        if src.space == bass.MemorySpace.PSUM:
            t = small.tile([128, 512], F32, tag="dbgps", name=name)
            nc.scalar.activation(out=t[:sh[0], :sh[1]], in_=src, func=ACTF.Copy)
            src = t[:sh[0], :sh[1]]
        nc.gpsimd.dma_start(out=io[name][:, :], in_=src)

    wpool = ctx.enter_context(tc.tile_pool(name="wpool", bufs=1))
    const = ctx.enter_context(tc.tile_pool(name="const", bufs=1))
    persist = ctx.enter_context(tc.tile_pool(name="persist", bufs=1))
    work = ctx.enter_context(tc.tile_pool(name="work", bufs=1))
    # u triple-buffers on hardware; drop to 2 in debug builds to make room
    # for the debug dump staging (timing is irrelevant in CoreSim)
    upool = ctx.enter_context(tc.tile_pool(name="upool", bufs=(2 if debug else 3)))
    vpool = ctx.enter_context(tc.tile_pool(name="vpool", bufs=1))
    small = ctx.enter_context(tc.tile_pool(name="small", bufs=2))

    pp_mm = ctx.enter_context(tc.tile_pool(name="pp_mm", bufs=2, space="PSUM"))
    pp_sim = ctx.enter_context(tc.tile_pool(name="pp_sim", bufs=1, space="PSUM"))
    pp_tr = ctx.enter_context(tc.tile_pool(name="pp_tr", bufs=2, space="PSUM"))
    pp_r = ctx.enter_context(tc.tile_pool(name="pp_r", bufs=1, space="PSUM"))
    pp_sm = ctx.enter_context(tc.tile_pool(name="pp_sm", bufs=2, space="PSUM"))

    # ---------------- constants ----------------
    identf = const.tile([128, 128], F32, tag="identf", name="identf")
    make_identity(nc, identf[:, :])
    identr = const.tile([128, 128], TRUNK, tag="identr", name="identr")
    nc.vector.tensor_scalar_add(out=identr[:, :], in0=identf[:, :], scalar1=0.0)
    identb = const.tile([128, 128], H16, tag="identb", name="identb")
    nc.vector.tensor_scalar_add(out=identb[:, :], in0=identf[:, :], scalar1=0.0)

    # sliding ones-column buffer: Z[:, 32] == 1 so Z[:, 32-r:64-r] has its
    # ones in column r; Z_slice.T @ U deposits column-sums of U into row r.
    zbuf = const.tile([128, 64], H16, tag="zbuf", name="zbuf")
    nc.vector.memset(zbuf[:, :], 0.0)
    nc.vector.memset(zbuf[:, 32:33], 1.0)

    neg1 = const.tile([128, 1], F32, tag="neg1", name="neg1")
    nc.vector.memset(neg1[:, :], -1.0)

    # ---------------- weights: casting DMAs via gpsimd queue --------------
    SPECIAL_KCH = {
        "cmp_W1": [(s * D + o, c) for s in range(4) for (o, c) in CH_D],
        "agg_W1": [(s * P + o, c) for s in range(4) for (o, c) in CH_P],
    }

    def load_w(name):
        h = io[name]
        K, M = h.shape
        H16_W = {"cmp_W1", "cmp_W2", "chw1_Wh", "chw1_Wt", "chw2_Wh", "chw2_Wt"}
        dt = F32 if name in F32_WEIGHTS else (H16 if name in H16_W else TRUNK)
        tiles = []
        for i, (o, c) in enumerate(SPECIAL_KCH.get(name, _chunks(K))):
            t = wpool.tile([c, M], dt, tag=f"w_{name}_{i}", name=f"w_{name}_{i}")
            eng = nc.sync if dt == F32 else nc.gpsimd
            eng.dma_start(out=t[:, :], in_=h[o:o + c, :])
            tiles.append(t)
        return tiles

    def load_b(name):
        h = io[name]
        (M,) = h.shape
        tiles = []
        for i, (o, c) in enumerate(_chunks(M)):
            t = wpool.tile([c, 1], F32, tag=f"b_{name}_{i}", name=f"b_{name}_{i}")
            nc.sync.dma_start(out=t[:, :], in_=h[o:o + c])
            tiles.append(t)
        return tiles

    # ---------------- index DMAs + gathers (overlap weight DMAs) ----------
    pre2 = ctx.enter_context(ExitStack())
    g2pool = pre2.enter_context(tc.tile_pool(name="g2pool", bufs=1))
    pre1 = ctx.enter_context(ExitStack())
    gpool = pre1.enter_context(tc.tile_pool(name="gpool", bufs=1))
    e_n = {}
    for side, xh in (("1", io["x1"]), ("2", io["x2"])):
        for b in range(BL):
            idx = gpool.tile([128, 1], I32, tag=f"idx{side}_{b}", name=f"idx{side}_{b}")
            nc.sync.dma_start(out=idx[:, :], in_=xh[b, :])
            e = gpool.tile([128, D], H16, tag=f"e{side}_{b}", name=f"e{side}_{b}")
            nc.gpsimd.indirect_dma_start(
                out=e[:, :], out_offset=None, in_=io["emb"][:, :],
                in_offset=bass.IndirectOffsetOnAxis(ap=idx[:, :1], axis=0),
            )
            e_n[(side, b)] = e

    W = {}
    for n in WEIGHT_NAMES:
        W[n] = load_b(n) if n.endswith(("bh", "bt", "b1", "b2", "_b")) else load_w(n)

    # ---------------- helpers ----------------
    def mm_apply(w_tiles, b_tiles, rhs_tiles, n_free, func, out_tiles,
                 krange=None, mrange=None):
        """out = func(W.T @ rhs + b), transposed layout, 512-col PSUM chunks."""
        M = w_tiles[0].shape[1]
        mch = _chunks(M)
        ks = list(range(len(w_tiles))) if krange is None else krange
        m_iter = ([(i, i) for i in range(len(mch))] if mrange is None
                  else list(enumerate(mrange)))
        for oi, mi in m_iter:
            mo, mc = mch[mi]
            for fo in range(0, n_free, 512):
                fc = min(512, n_free - fo)
                ps = pp_mm.tile([128, 512], F32, tag="mmout", name="mmout")
                for idx, ki in enumerate(ks):
                    kc = w_tiles[ki].shape[0]
                    nc.tensor.matmul(
                        out=ps[:mc, :fc],
                        lhsT=w_tiles[ki][:kc, mo:mo + mc],
                        rhs=rhs_tiles[ki][:kc, fo:fo + fc],
                        start=(idx == 0),
                        stop=(idx == len(ks) - 1),
                    )
                nc.scalar.activation(
                    out=out_tiles[oi][:mc, fo:fo + fc],
                    in_=ps[:mc, :fc],
                    func=func, bias=b_tiles[mi][:mc, :], scale=1.0,
                )

    def highway(xt_tiles, wh, bh, wt, bt, feat, out_tiles):
        """out = x + t*(h-x), trunk layout, chunk-at-a-time (h reused as tmp)."""
        ch = _chunks(feat)
        for mi, (mo, mc) in enumerate(ch):
            h = work.tile([128, ROWS2], TRUNK, tag="hw_h", name="hw_h")
            t = work.tile([128, ROWS2], TRUNK, tag="hw_t", name="hw_t")
            mm_apply(wh, bh, xt_tiles, ROWS2, ACTF.Relu, [h], mrange=[mi])
            mm_apply(wt, bt, xt_tiles, ROWS2, ACTF.Sigmoid, [t], mrange=[mi])
            x_sl = xt_tiles[mi][:mc, :]
            nc.vector.tensor_tensor(out=h[:mc, :], in0=h[:mc, :], in1=x_sl,
                                    op=ALU.subtract)
            nc.vector.tensor_tensor(out=h[:mc, :], in0=h[:mc, :], in1=t[:mc, :],
                                    op=ALU.mult)
            nc.vector.tensor_tensor(out=out_tiles[mi][:mc, :], in0=h[:mc, :],
                                    in1=x_sl, op=ALU.add)

    # ---------------- embed: transpose into trunk ----------------
    # eT[ki]: [kc, 1024], col = side*512 + b*128 + token
    eT = [g2pool.tile([128, ROWS2], TRUNK, tag=f"eT_{i}", name=f"eT_{i}")
          for i in range(3)]
    for ki, (ko, kc) in enumerate(CH_D):
        for side in ("1", "2"):
            ps = pp_tr.tile([128, 512], H16, tag="trpackb", name="trpack")
            for b in range(BL):
                nc.tensor.transpose(
                    out=ps[:kc, b * S:(b + 1) * S],
                    in_=e_n[(side, b)][:, ko:ko + kc],
                    identity=identb[:128, :128],
                )
            so = (0 if side == "1" else ROWS)
            nc.scalar.activation(out=eT[ki][:kc, so:so + ROWS], in_=ps[:kc, :ROWS],
                                 func=ACTF.Copy)
    pre1.close()  # frees index + gather tiles

    # ---------------- highway stack (trunk: both sides at once) -------------
    h1 = [g2pool.tile([128, ROWS2], TRUNK, tag=f"hwy1_{i}", name=f"hwy1_{i}")
          for i in range(3)]
    highway(eT, W["hw1_Wh"], W["hw1_bh"], W["hw1_Wt"], W["hw1_bt"], D, h1)
    eTh = [persist.tile([128, ROWS2], TRUNK, tag=f"eTh_{i}", name=f"eTh_{i}")
           for i in range(3)]
    highway(h1, W["hw2_Wh"], W["hw2_bh"], W["hw2_Wt"], W["hw2_bt"], D, eTh)
    pre2.close()  # frees eT, h1

    # normal-layout post-highway embeddings (lhsT for the beta/alpha matmuls)
    ehw_n = {}
    for side in ("1", "2"):
        so = (0 if side == "1" else ROWS)
        for b in range(BL):
            ps = pp_r.tile([128, 512], TRUNK, tag="trpackr", name="trpackr")
            for ki, (ko, kc) in enumerate(CH_D):
                nc.tensor.transpose(
                    out=ps[:128, ko:ko + kc],
                    in_=eTh[ki][:kc, so + b * S:so + (b + 1) * S],
                    identity=identr[:kc, :kc],
                )
            t = persist.tile([128, D], H16, tag=f"ehwn{side}_{b}", name=f"ehwn{side}_{b}")
            nc.scalar.activation(out=t[:, :], in_=ps[:, :D], func=ACTF.Copy)
            ehw_n[(side, b)] = t

    # ---------------- projections (shared weights, trunk) ----------------
    def proj(prefix, pool):
        z1 = [work.tile([128, ROWS2], TRUNK, tag=f"z1_{i}", name=f"z1_{i}") for i in range(2)]
        mm_apply(W[f"{prefix}_W1"], W[f"{prefix}_b1"], eTh, ROWS2, ACTF.Relu, z1)
        out = [pool.tile([128, ROWS2], TRUNK, tag=f"{prefix}T_{i}", name=f"{prefix}T_{i}")
               for i in range(2)]
        mm_apply(W[f"{prefix}_W2"], W[f"{prefix}_b2"], z1, ROWS2, ACTF.Relu, out)
        return out

    # dist first so the att2 elementwise can start while the PE continues
    # with the mul projection (att1 is only needed at b0's first fold_sum)
    qscope = ctx.enter_context(ExitStack())
    qpool = qscope.enter_context(tc.tile_pool(name="qpool", bufs=1))
    qT = proj("dist", qpool)
    dbg_dump("dbg_eTh0", eTh[0][:, :])
    dbg_dump("dbg_qT0", qT[0][:, :])

    # fp16 views of q for the att2 elementwise: q1p = q1+1, q2b = q2.
    # The unused partition rows 72:128 of the low chunk are zeroed so the
    # subtract can run over all 128 partitions without stale data.
    q1p, q2b = [], []
    for ki in range(2):
        kc = CH_P[ki][1]
        tp = persist.tile([128, ROWS], H16, tag=f"q1p_{ki}", name=f"q1p_{ki}")
        t2 = persist.tile([128, ROWS], H16, tag=f"q2b_{ki}", name=f"q2b_{ki}")
        if kc < 128:
            nc.vector.memset(tp[64:128, :], 0.0)
            nc.vector.memset(t2[64:128, :], 0.0)
        nc.vector.tensor_scalar_add(out=tp[:kc, :], in0=qT[ki][:kc, :ROWS], scalar1=1.0)
        nc.vector.tensor_scalar_add(out=t2[:kc, :], in0=qT[ki][:kc, ROWS:], scalar1=0.0)
        q1p.append(tp)
        q2b.append(t2)
    qscope.close()  # frees qT

    pT = proj("mul", persist)

    # ---------------- att1 into sim4 PSUM (simT layout [j, i] per b) --------
    # start=True lazily marks the WHOLE bank pending-zero, so only the very
    # first matmul touching sim4 carries it; every later first-touch of a byte
    # overwrites, and overlapping writes accumulate. The att2 sums then
    # accumulate on top with start=False.
    sim4 = pp_sim.tile([128, 512], F32, tag="sim4", name="sim4")
    for b in range(BL):
        for ki, (ko, kc) in enumerate(CH_P):
            nc.tensor.matmul(
                out=sim4[:, b * S:(b + 1) * S],
                lhsT=pT[ki][:kc, ROWS + b * S:ROWS + (b + 1) * S],
                rhs=pT[ki][:kc, b * S:(b + 1) * S],
                start=(b == 0 and ki == 0), stop=False, skip_group_check=True,
            )

    # ---------------- att2: dist attention ----------------
    # Each j-block covers ALL 4 batch items: u layout [p, (b, j, i)]. The
    # subtracts run as small per-(b, chunk) instructions (good pipelining);
    # the rest of the elementwise runs whole-tile; the partition-sum matmuls
    # take a b-strided 512-wide rhs per j and write all of sim4's rows.
    half = JB * 512  # hi/lo chunk size in u (b, j, i)

    def att2_block(jb, type_a):
        u = upool.tile([128, 2 * half], H16, tag="u", name="u")
        for ki, off in ((0, 0), (1, half)):
            for b in range(BL):
                uo = u[:128, off + b * JB * S:off + (b + 1) * JB * S].rearrange(
                    "p (j i) -> p j i", j=JB)
                q1s = q1p[ki][:128, b * S:(b + 1) * S]
                in0 = bass.AP(tensor=q1s.tensor, offset=q1s.offset,
                              ap=[q1s.ap[0], [0, JB], q1s.ap[1]])
                q2s = q2b[ki][:128, b * S + jb * JB:b * S + (jb + 1) * JB]
                in1 = bass.AP(tensor=q2s.tensor, offset=q2s.offset,
                              ap=[q2s.ap[0], q2s.ap[1], [0, S]])
                nc.vector.tensor_tensor(out=uo, in0=in0, in1=in1,
                                        op=ALU.subtract)
        if type_a:
            # v = 2-u = 1-x ; u = max(u,v) = 1+|x| ; u = 1/u
            v = vpool.tile([128, 2 * half], H16, tag="v", name="v")
            nc.vector.tensor_scalar(out=v[:, :], in0=u[:, :], scalar1=-1.0,
                                    scalar2=2.0, op0=ALU.mult, op1=ALU.add)
            nc.vector.tensor_tensor(out=u[:, :], in0=u[:, :], in1=v[:, :],
                                    op=ALU.max)
            act_recip(nc, u[:, :], u[:, :], bias=0.0)
        else:
            # u = |u-1| = |x| ; u = 1/(1+u)
            nc.scalar.activation(out=u[:, :], in_=u[:, :], func=ACTF.Abs,
                                 bias=neg1[:, :])
            act_recip(nc, u[:, :], u[:, :], bias=1.0)
        return u

    def att2_fold_sum(jb, u):
        # fold the p=128..200 chunk onto the first 72 rows of the hi chunk,
        # alternating DVE with the otherwise-idle (but slow) gpsimd
        eng = nc.vector if jb % 2 == 0 else nc.gpsimd
        eng.tensor_tensor(out=u[:72, :half], in0=u[:72, :half],
                          in1=u[:72, half:], op=ALU.add)
        for jj in range(JB):
            j = jb * JB + jj
            g, rr = j // 32, j % 32
            rbase = u[:128, jj * S:jj * S + S]
            rhs = bass.AP(tensor=rbase.tensor, offset=rbase.offset,
                          ap=[rbase.ap[0], [JB * S, BL], [1, S]])
            nc.tensor.matmul(
                out=sim4[32 * g:32 * g + 32, :],
                lhsT=zbuf[:128, 32 - rr:64 - rr],
                rhs=rhs,
                start=False, stop=(rr == 31), skip_group_check=True,
                tile_position=(0, 32 * g),
            )

    prev = None
    for jb in range(NBLK):
        type_a = (jb * A_NUM) % A_DEN < A_NUM
        u = att2_block(jb, type_a)
        if prev is not None:
            att2_fold_sum(*prev)
        prev = (jb, u)
    att2_fold_sum(*prev)
    dbg_dump("dbg_sim4", sim4[:, :])

    # ---------------- softmax + beta/alpha + compare part 1 ----------------
    def softmax_p(src_psum):
        """softmax over rows of src [128,128]; returns transposed probs bf16."""
        mx = small.tile([128, 1], F32, tag="sm_mx", name="sm_mx")
        nc.vector.tensor_reduce(out=mx[:, :], in_=src_psum, axis=AX.X,
                                op=ALU.max, negate=True)
        esb = small.tile([128, S], H16, tag="sm_e", name="sm_e")
        zs = small.tile([128, 1], F32, tag="sm_z", name="sm_z")
        nc.scalar.activation(out=esb[:, :], in_=src_psum, func=ACTF.Exp,
                             bias=mx[:, :], scale=1.0, accum_out=zs[:, :])
        rz = small.tile([128, 1], F32, tag="sm_rz", name="sm_rz")
        nc.vector.reciprocal(out=rz[:, :], in_=zs[:, :])
        pr = small.tile([128, S], H16, tag="sm_p", name="sm_p")
        nc.vector.tensor_scalar(out=pr[:, :], in0=esb[:, :], scalar1=rz[:, :],
                                scalar2=None, op0=ALU.mult)
        pt_ps = pp_tr.tile([128, 512], H16, tag="trpackb", name="trpackb")
        nc.tensor.transpose(out=pt_ps[:S, :S], in_=pr[:, :], identity=identb[:, :])
        pt = small.tile([128, S], H16, tag="sm_pt", name="sm_pt")
        nc.scalar.activation(out=pt[:, :], in_=pt_ps[:S, :S], func=ACTF.Copy)
        return pt

    # betaT trunk tiles per side: [kc, 512] bf16, col = b*128 + token
    betaT = {s: [persist.tile([128, 512], H16, tag=f"betaT{s}_{i}", name=f"betaT{s}_{i}")
                 for i in range(3)] for s in ("1", "2")}

    for b in range(BL):
        bs4 = sim4[:, b * S:(b + 1) * S]
        ptA = softmax_p(bs4)  # alpha probs^T [i, j]
        simT_sb = small.tile([128, S], F32, tag="simT_sb", name="simT_sb")
        nc.scalar.activation(out=simT_sb[:, :], in_=bs4, func=ACTF.Copy)
        sim_ps = pp_sm.tile([128, S], F32, tag="btps", name="simtr")
        nc.tensor.transpose(out=sim_ps[:S, :S], in_=simT_sb[:, :],
                            identity=identf[:, :])
        ptB = softmax_p(sim_ps[:S, :S])  # beta probs^T [j, i]

        for side, pt, eln in (("1", ptB, "2"), ("2", ptA, "1")):
            for ki, (ko, kc) in enumerate(CH_D):
                bt_ps = pp_sm.tile([128, S], F32, tag="btps", name="btps")
                nc.tensor.matmul(
                    out=bt_ps[:kc, :], lhsT=ehw_n[(eln, b)][:, ko:ko + kc],
                    rhs=pt[:, :], start=True, stop=True,
                )
                nc.scalar.activation(
                    out=betaT[side][ki][:kc, b * S:(b + 1) * S],
                    in_=bt_ps[:kc, :], func=ACTF.Copy)

    dbg_dump("dbg_betaT0", betaT["1"][0][:, :])

    # cat + compare matmul, per side over 512-col trunk halves. The cat
    # chunks (e-b, e*b) are computed on the fly right before their two
    # accumulating matmuls, so only 2 transient tiles are alive at a time.
    cmp1 = [persist.tile([128, ROWS2], H16, tag=f"cmp1_{i}", name=f"cmp1_{i}")
            for i in range(2)]
    for side in ("1", "2"):
        so = (0 if side == "1" else ROWS)
        ps2 = [pp_mm.tile([128, 512], F32, tag="mmout", name=f"cmp1ps{mi}")
               for mi in range(2)]
        for sel in range(4):  # e, beta, e-beta, e*beta
            for ki, (ko, kc) in enumerate(CH_D):
                e_sl = eTh[ki][:kc, so:so + ROWS]
                b_sl = betaT[side][ki][:kc, :]
                if sel == 1:
                    rhs = b_sl
                else:
                    cat = small.tile([128, 512], H16, tag="cat", name="cat")
                    if sel == 0:
                        nc.vector.tensor_scalar_add(out=cat[:kc, :], in0=e_sl,
                                                    scalar1=0.0)
                    else:
                        nc.vector.tensor_tensor(
                            out=cat[:kc, :], in0=e_sl, in1=b_sl,
                            op=(ALU.subtract if sel == 2 else ALU.mult))
                    rhs = cat[:kc, :]
                idx = sel * 3 + ki
                for mi, (mo, mc) in enumerate(CH_P):
                    nc.tensor.matmul(
                        out=ps2[mi][:mc, :],
                        lhsT=W["cmp_W1"][idx][:, mo:mo + mc],
                        rhs=rhs,
                        start=(idx == 0), stop=(idx == 11),
                        skip_group_check=True,
                    )
        for mi, (mo, mc) in enumerate(CH_P):
            nc.scalar.activation(
                out=cmp1[mi][:mc, so:so + ROWS], in_=ps2[mi][:mc, :],
                func=ACTF.Relu,
                bias=W["cmp_b1"][mi][:mc, :], scale=1.0,
            )

    # ---------------- compare part 2 + compare highway (trunk) --------------
    v0 = [work.tile([128, ROWS2], H16, tag=f"v0_{i}", name=f"v0_{i}") for i in range(2)]
    mm_apply(W["cmp_W2"], W["cmp_b2"], cmp1, ROWS2, ACTF.Relu, v0)
    v1 = [work.tile([128, ROWS2], H16, tag=f"v1_{i}", name=f"v1_{i}") for i in range(2)]
    highway(v0, W["chw1_Wh"], W["chw1_bh"], W["chw1_Wt"], W["chw1_bt"], P, v1)
    vT = [persist.tile([128, ROWS2], H16, tag=f"vT_{i}", name=f"vT_{i}")
          for i in range(2)]
    highway(v1, W["chw2_Wh"], W["chw2_bh"], W["chw2_Wt"], W["chw2_bt"], P, vT)
    dbg_dump("dbg_vT0", vT[0][:, :])

    # ---------------- aggregate (fp32) ----------------
    # stats[sect][ki]: [kc, BL]; sections: v1.max, v2.max, v1.sum, v2.sum
    stats = []
    for sect, (side, op) in enumerate(
            (("1", ALU.max), ("2", ALU.max), ("1", ALU.add), ("2", ALU.add))):
        so = (0 if side == "1" else ROWS)
        st = [persist.tile([128, BL], F32, tag=f"st{sect}_{i}", name=f"st{sect}_{i}")
              for i in range(2)]
        for ki, (ko, kc) in enumerate(CH_P):
            seg = vT[ki][:kc, so:so + ROWS].rearrange("p (b t) -> p b t", b=BL)
            nc.vector.tensor_reduce(
                out=st[ki][:kc, :BL], in_=seg, axis=AX.X, op=op,
            )
        stats.append(st)

    agg_rhs = [stats[s][ki] for s in range(4) for ki in range(2)]
    y1 = [persist.tile([128, BL], F32, tag=f"y1_{i}", name=f"y1_{i}") for i in range(2)]
    mm_apply(W["agg_W1"], W["agg_b1"], agg_rhs, BL, ACTF.Relu, y1)
    y2 = [persist.tile([128, BL], F32, tag=f"y2_{i}", name=f"y2_{i}") for i in range(2)]
    mm_apply(W["agg_W2"], W["agg_b2"], y1, BL, ACTF.Relu, y2)

    yt_ps = pp_sm.tile([128, S], F32, tag="btps", name="btps")
    for ki, (ko, kc) in enumerate(CH_P):
        nc.tensor.matmul(
            out=yt_ps[:C, :BL], lhsT=W["out_W"][ki][:kc, :],
            rhs=y2[ki][:kc, :], start=(ki == 0), stop=(ki == 1),
        )
    yt_sb = persist.tile([C, BL], F32, tag="yt_sb", name="yt_sb")
    nc.scalar.activation(out=yt_sb[:, :], in_=yt_ps[:C, :BL], func=ACTF.Identity,
                         bias=W["out_b"][0][:C, :], scale=1.0)
    nc.sync.dma_start(out=io["yt"][:, :], in_=yt_sb[:, :])


_NC_CACHE = {}


def _get_nc():
    if "nc" not in _NC_CACHE:
        _NC_CACHE["nc"] = build_nc()
    return _NC_CACHE["nc"]


def make_in_maps(inputs):
    """Shard full inputs into 8 per-core input maps."""
    x1 = np.ascontiguousarray(np.asarray(inputs["x1"]).astype(np.int32))
    x2 = np.ascontiguousarray(np.asarray(inputs["x2"]).astype(np.int32))
    shared = {}
    for n in WEIGHT_NAMES + ["emb"]:
        shared[n] = np.ascontiguousarray(np.asarray(inputs[n]).astype(np.float32))
    in_maps = []
    for c in range(NCORES):
        m = dict(shared)
        m["x1"] = x1[c * BL:(c + 1) * BL]
        m["x2"] = x2[c * BL:(c + 1) * BL]
        in_maps.append(m)
    return in_maps


def kernel(**inputs):
    nc = _get_nc()
    in_maps = make_in_maps(inputs)
    res = run_bass_kernel_spmd(nc, in_maps, core_ids=list(range(NCORES)))
    return np.concatenate([np.asarray(r["yt"]).T for r in res.results], axis=0)


if __name__ == "__main__":
    nc = build_nc()
    print("built ok")


# revision 71
# speedup vs baseline: 1.4990x; 1.1824x over previous
"""Trainium2 Bass kernel for nn_AttentiveModel (B=32,S=128,D=300,P=200,V=30000,C=3).

Data-parallel over batch across 8 NeuronCores (4 batch items per core, all
weights replicated). Trunk compute (highways/projections/compare) runs in
float32r on the PE (1 cycle/row at free>=256, near-fp32 precision); the
dist-attention elementwise runs in bf16 split across DVE and ScalarE.

Layout: activations live transposed [features(partitions), rows(free)] with
both sides sharing one 1024-col trunk (col = side*512 + b*128 + token), so
every shared-weight matmul/elementwise runs once over both sides.

att2[b,j,i] = sum_p 1/(1+|q1[b,i,p]-q2[b,j,p]|), streamed in j-blocks:
  type-A blocks (DVE abs):  w=(q1+1)-q2 (TT), v=2-w (TS 4x),
                            s=max(w,v)=1+|x| (TT), r=1/s (ScalarE Reciprocal)
  type-B blocks (ScalarE abs): d=q1-q2 (TT), |d| (ScalarE Abs),
                            r=1/(1+|d|) (ScalarE Reciprocal bias=1)
  then a DVE fold of the p=128..200 chunk onto the first 72 rows and a
  partition-sum via PE matmuls with a sliding ones-column lhsT accumulating
  into the sim PSUM tile on top of att1.
ScalarE Reciprocal is emitted directly as InstActivation (bass's wrapper
refuses it on accuracy grounds far below this problem's 2e-2 tolerance).
"""

import sys
from contextlib import ExitStack

import numpy as np

for _p in ("/opt/trn_rl_repo",):
    if _p not in sys.path:
        sys.path.insert(0, _p)

import concourse.bass as bass
import concourse.tile as tile
from concourse.bacc import Bacc
from concourse import mybir
from concourse.bass_utils import run_bass_kernel_spmd
from concourse.masks import make_identity

F32 = mybir.dt.float32
F32R = mybir.dt.float32r
BF = mybir.dt.bfloat16
H16 = mybir.dt.float16
I32 = mybir.dt.int32
ALU = mybir.AluOpType
ACTF = mybir.ActivationFunctionType
AX = mybir.AxisListType

TRUNK = F32R  # trunk compute dtype (flip to H16 to trade accuracy for SBUF)

B, S, D, P, V, C = 32, 128, 300, 200, 30000, 3
NCORES = 8
BL = B // NCORES  # 4 batch items per core
ROWS = BL * S  # 512 per side
ROWS2 = 2 * ROWS  # both sides in one trunk

CH_D = [(0, 128), (128, 128), (256, 44)]  # 300
CH_P = [(0, 128), (128, 72)]  # 200

JB = 8  # j-block size for att2 streaming (each block covers all 4 b)
NBLK = S // JB
# fraction of j-blocks whose abs runs on DVE (type A) vs ScalarE (type B)
A_NUM, A_DEN = 1, 5

WEIGHT_NAMES = [
    "hw1_Wh", "hw1_bh", "hw1_Wt", "hw1_bt",
    "hw2_Wh", "hw2_bh", "hw2_Wt", "hw2_bt",
    "mul_W1", "mul_b1", "mul_W2", "mul_b2",
    "dist_W1", "dist_b1", "dist_W2", "dist_b2",
    "cmp_W1", "cmp_b1", "cmp_W2", "cmp_b2",
    "chw1_Wh", "chw1_bh", "chw1_Wt", "chw1_bt",
    "chw2_Wh", "chw2_bh", "chw2_Wt", "chw2_bt",
    "agg_W1", "agg_b1", "agg_W2", "agg_b2",
    "out_W", "out_b",
]

# weights kept fp32 (tiny free dims in the aggregate MLP)
F32_WEIGHTS = {"agg_W1", "agg_W2", "out_W"}


def _chunks(n):
    out = []
    o = 0
    while o < n:
        c = min(128, n - o)
        out.append((o, c))
        o += c
    return out


def act_recip(nc, out, in_, bias=0.0):
    """out = 1/(in_ + bias) in one ScalarE pass (Reciprocal activation)."""
    eng = nc.scalar
    ins_ = [
        eng.lower_ap(in_),
        mybir.ImmediateValue(dtype=mybir.dt.float32, value=bias),  # bias
        mybir.ImmediateValue(dtype=mybir.dt.float32, value=1.0),  # scale
        mybir.ImmediateValue(dtype=mybir.dt.float32, value=0.0),  # alpha
    ]
    return eng.add_instruction(
        mybir.InstActivation(
            name=eng.bass.get_next_instruction_name(),
            func=ACTF.Reciprocal,
            ins=ins_,
            outs=[eng.lower_ap(out)],
        )
    )


def build_nc(debug=False):
    nc = Bacc()

    io = {}
    io["x1"] = nc.declare_dram_parameter("x1", [BL, S], I32, isOutput=False)
    io["x2"] = nc.declare_dram_parameter("x2", [BL, S], I32, isOutput=False)
    io["emb"] = nc.declare_dram_parameter("emb", [V, D], F32, isOutput=False)
    shapes = {
        "hw1_Wh": [D, D], "hw1_bh": [D], "hw1_Wt": [D, D], "hw1_bt": [D],
        "hw2_Wh": [D, D], "hw2_bh": [D], "hw2_Wt": [D, D], "hw2_bt": [D],
        "mul_W1": [D, P], "mul_b1": [P], "mul_W2": [P, P], "mul_b2": [P],
        "dist_W1": [D, P], "dist_b1": [P], "dist_W2": [P, P], "dist_b2": [P],
        "cmp_W1": [4 * D, P], "cmp_b1": [P], "cmp_W2": [P, P], "cmp_b2": [P],
        "chw1_Wh": [P, P], "chw1_bh": [P], "chw1_Wt": [P, P], "chw1_bt": [P],
        "chw2_Wh": [P, P], "chw2_bh": [P], "chw2_Wt": [P, P], "chw2_bt": [P],
        "agg_W1": [4 * P, P], "agg_b1": [P], "agg_W2": [P, P], "agg_b2": [P],
        "out_W": [P, C], "out_b": [C],
    }
    for n in WEIGHT_NAMES:
        io[n] = nc.declare_dram_parameter(n, shapes[n], F32, isOutput=False)
    io["yt"] = nc.declare_dram_parameter("yt", [C, BL], F32, isOutput=True)
    if debug:
        io["dbg_eTh0"] = nc.declare_dram_parameter("dbg_eTh0", [128, ROWS2], F32, isOutput=True)
        io["dbg_qT0"] = nc.declare_dram_parameter("dbg_qT0", [128, ROWS2], F32, isOutput=True)
        io["dbg_sim4"] = nc.declare_dram_parameter("dbg_sim4", [128, 512], F32, isOutput=True)
        io["dbg_betaT0"] = nc.declare_dram_parameter("dbg_betaT0", [128, 512], F32, isOutput=True)
        io["dbg_vT0"] = nc.declare_dram_parameter("dbg_vT0", [128, ROWS2], F32, isOutput=True)

    with ExitStack() as ctx:
        tc = ctx.enter_context(tile.TileContext(nc))
        _emit(ctx, nc, tc, io, debug=debug)
    nc.finalize()
    return nc


def _emit(ctx, nc, tc, io, debug=False):
    def dbg_dump(name, ap):
        if not debug or name not in io:
            return
        sh = io[name].shape
        src = ap[:sh[0], :sh[1]]
        if src.space == bass.MemorySpace.PSUM:
            t = small.tile([128, 512], F32, tag="dbgps", name=name)
            nc.scalar.activation(out=t[:sh[0], :sh[1]], in_=src, func=ACTF.Copy)
            src = t[:sh[0], :sh[1]]
        nc.gpsimd.dma_start(out=io[name][:, :], in_=src)

    wpool = ctx.enter_context(tc.tile_pool(name="wpool", bufs=1))
    const = ctx.enter_context(tc.tile_pool(name="const", bufs=1))
    persist = ctx.enter_context(tc.tile_pool(name="persist", bufs=1))
    work = ctx.enter_context(tc.tile_pool(name="work", bufs=1))
    # u triple-buffers on hardware; drop to 2 in debug builds to make room
    # for the debug dump staging (timing is irrelevant in CoreSim)
    upool = ctx.enter_context(tc.tile_pool(name="upool", bufs=(2 if debug else 3)))
    vpool = ctx.enter_context(tc.tile_pool(name="vpool", bufs=1))
    small = ctx.enter_context(tc.tile_pool(name="small", bufs=2))

    pp_mm = ctx.enter_context(tc.tile_pool(name="pp_mm", bufs=2, space="PSUM"))
    pp_sim = ctx.enter_context(tc.tile_pool(name="pp_sim", bufs=1, space="PSUM"))
    pp_tr = ctx.enter_context(tc.tile_pool(name="pp_tr", bufs=2, space="PSUM"))
    pp_r = ctx.enter_context(tc.tile_pool(name="pp_r", bufs=1, space="PSUM"))
    pp_sm = ctx.enter_context(tc.tile_pool(name="pp_sm", bufs=2, space="PSUM"))

    # ---------------- constants ----------------
    identf = const.tile([128, 128], F32, tag="identf", name="identf")
    make_identity(nc, identf[:, :])
    identr = const.tile([128, 128], TRUNK, tag="identr", name="identr")
    nc.vector.tensor_scalar_add(out=identr[:, :], in0=identf[:, :], scalar1=0.0)
    identb = const.tile([128, 128], H16, tag="identb", name="identb")
    nc.vector.tensor_scalar_add(out=identb[:, :], in0=identf[:, :], scalar1=0.0)

    # sliding ones-column buffer: Z[:, 32] == 1 so Z[:, 32-r:64-r] has its
    # ones in column r; Z_slice.T @ U deposits column-sums of U into row r.
    zbuf = const.tile([128, 64], H16, tag="zbuf", name="zbuf")
    nc.vector.memset(zbuf[:, :], 0.0)
    nc.vector.memset(zbuf[:, 32:33], 1.0)

    neg1 = const.tile([128, 1], F32, tag="neg1", name="neg1")
    nc.vector.memset(neg1[:, :], -1.0)

    # ---------------- weights: casting DMAs via gpsimd queue --------------
    SPECIAL_KCH = {
        "cmp_W1": [(s * D + o, c) for s in range(4) for (o, c) in CH_D],
        "agg_W1": [(s * P + o, c) for s in range(4) for (o, c) in CH_P],
    }

    def load_w(name):
        h = io[name]
        K, M = h.shape
        H16_W = {"cmp_W1", "cmp_W2", "chw1_Wh", "chw1_Wt", "chw2_Wh", "chw2_Wt"}
        dt = F32 if name in F32_WEIGHTS else (H16 if name in H16_W else TRUNK)
        tiles = []
        for i, (o, c) in enumerate(SPECIAL_KCH.get(name, _chunks(K))):
            t = wpool.tile([c, M], dt, tag=f"w_{name}_{i}", name=f"w_{name}_{i}")
            eng = nc.sync if dt == F32 else nc.gpsimd
            eng.dma_start(out=t[:, :], in_=h[o:o + c, :])
            tiles.append(t)
        return tiles

    def load_b(name):
        h = io[name]
        (M,) = h.shape
        tiles = []
        for i, (o, c) in enumerate(_chunks(M)):
            t = wpool.tile([c, 1], F32, tag=f"b_{name}_{i}", name=f"b_{name}_{i}")
            nc.sync.dma_start(out=t[:, :], in_=h[o:o + c])
            tiles.append(t)
        return tiles

    # ---------------- index DMAs + gathers (overlap weight DMAs) ----------
    pre2 = ctx.enter_context(ExitStack())
    g2pool = pre2.enter_context(tc.tile_pool(name="g2pool", bufs=1))
    pre1 = ctx.enter_context(ExitStack())
    gpool = pre1.enter_context(tc.tile_pool(name="gpool", bufs=1))
    e_n = {}
    for side, xh in (("1", io["x1"]), ("2", io["x2"])):
        for b in range(BL):
            idx = gpool.tile([128, 1], I32, tag=f"idx{side}_{b}", name=f"idx{side}_{b}")
            nc.sync.dma_start(out=idx[:, :], in_=xh[b, :])
            e = gpool.tile([128, D], H16, tag=f"e{side}_{b}", name=f"e{side}_{b}")
            nc.gpsimd.indirect_dma_start(
                out=e[:, :], out_offset=None, in_=io["emb"][:, :],
                in_offset=bass.IndirectOffsetOnAxis(ap=idx[:, :1], axis=0),
            )
            e_n[(side, b)] = e

    W = {}
    for n in WEIGHT_NAMES:
        W[n] = load_b(n) if n.endswith(("bh", "bt", "b1", "b2", "_b")) else load_w(n)

    # ---------------- helpers ----------------
    def mm_apply(w_tiles, b_tiles, rhs_tiles, n_free, func, out_tiles,
                 krange=None, mrange=None):
        """out = func(W.T @ rhs + b), transposed layout, 512-col PSUM chunks."""
        M = w_tiles[0].shape[1]
        mch = _chunks(M)
        ks = list(range(len(w_tiles))) if krange is None else krange
        m_iter = ([(i, i) for i in range(len(mch))] if mrange is None
                  else list(enumerate(mrange)))
        for oi, mi in m_iter:
            mo, mc = mch[mi]
            for fo in range(0, n_free, 512):
                fc = min(512, n_free - fo)
                ps = pp_mm.tile([128, 512], F32, tag="mmout", name="mmout")
                for idx, ki in enumerate(ks):
                    kc = w_tiles[ki].shape[0]
                    nc.tensor.matmul(
                        out=ps[:mc, :fc],
                        lhsT=w_tiles[ki][:kc, mo:mo + mc],
                        rhs=rhs_tiles[ki][:kc, fo:fo + fc],
                        start=(idx == 0),
                        stop=(idx == len(ks) - 1),
                    )
                nc.scalar.activation(
                    out=out_tiles[oi][:mc, fo:fo + fc],
                    in_=ps[:mc, :fc],
                    func=func, bias=b_tiles[mi][:mc, :], scale=1.0,
                )

    def highway(xt_tiles, wh, bh, wt, bt, feat, out_tiles):
        """out = x + t*(h-x), trunk layout, chunk-at-a-time (h reused as tmp)."""
        ch = _chunks(feat)
        for mi, (mo, mc) in enumerate(ch):
            h = work.tile([128, ROWS2], TRUNK, tag="hw_h", name="hw_h")
            t = work.tile([128, ROWS2], TRUNK, tag="hw_t", name="hw_t")
            mm_apply(wh, bh, xt_tiles, ROWS2, ACTF.Relu, [h], mrange=[mi])
            mm_apply(wt, bt, xt_tiles, ROWS2, ACTF.Sigmoid, [t], mrange=[mi])
            x_sl = xt_tiles[mi][:mc, :]
            nc.vector.tensor_tensor(out=h[:mc, :], in0=h[:mc, :], in1=x_sl,
                                    op=ALU.subtract)
            nc.vector.tensor_tensor(out=h[:mc, :], in0=h[:mc, :], in1=t[:mc, :],
                                    op=ALU.mult)
            nc.vector.tensor_tensor(out=out_tiles[mi][:mc, :], in0=h[:mc, :],
                                    in1=x_sl, op=ALU.add)

    # ---------------- embed: transpose into trunk ----------------
    # eT[ki]: [kc, 1024], col = side*512 + b*128 + token
    eT = [g2pool.tile([128, ROWS2], TRUNK, tag=f"eT_{i}", name=f"eT_{i}")
          for i in range(3)]
    for ki, (ko, kc) in enumerate(CH_D):
        for side in ("1", "2"):
            ps = pp_tr.tile([128, 512], H16, tag="trpackb", name="trpack")
            for b in range(BL):
                nc.tensor.transpose(
                    out=ps[:kc, b * S:(b + 1) * S],
                    in_=e_n[(side, b)][:, ko:ko + kc],
                    identity=identb[:128, :128],
                )
            so = (0 if side == "1" else ROWS)
            nc.scalar.activation(out=eT[ki][:kc, so:so + ROWS], in_=ps[:kc, :ROWS],
                                 func=ACTF.Copy)
    pre1.close()  # frees index + gather tiles

    # ---------------- highway stack (trunk: both sides at once) -------------
    h1 = [g2pool.tile([128, ROWS2], TRUNK, tag=f"hwy1_{i}", name=f"hwy1_{i}")
          for i in range(3)]
    highway(eT, W["hw1_Wh"], W["hw1_bh"], W["hw1_Wt"], W["hw1_bt"], D, h1)
    eTh = [persist.tile([128, ROWS2], TRUNK, tag=f"eTh_{i}", name=f"eTh_{i}")
           for i in range(3)]
    highway(h1, W["hw2_Wh"], W["hw2_bh"], W["hw2_Wt"], W["hw2_bt"], D, eTh)
    pre2.close()  # frees eT, h1

    # normal-layout post-highway embeddings (lhsT for the beta/alpha matmuls)
    ehw_n = {}
    for side in ("1", "2"):
        so = (0 if side == "1" else ROWS)
        for b in range(BL):
            ps = pp_r.tile([128, 512], TRUNK, tag="trpackr", name="trpackr")
            for ki, (ko, kc) in enumerate(CH_D):
                nc.tensor.transpose(
                    out=ps[:128, ko:ko + kc],
                    in_=eTh[ki][:kc, so + b * S:so + (b + 1) * S],
                    identity=identr[:kc, :kc],
                )
            t = persist.tile([128, D], H16, tag=f"ehwn{side}_{b}", name=f"ehwn{side}_{b}")
            nc.scalar.activation(out=t[:, :], in_=ps[:, :D], func=ACTF.Copy)
            ehw_n[(side, b)] = t

    # ---------------- projections (shared weights, trunk) ----------------
    def proj(prefix, pool):
        z1 = [work.tile([128, ROWS2], TRUNK, tag=f"z1_{i}", name=f"z1_{i}") for i in range(2)]
        mm_apply(W[f"{prefix}_W1"], W[f"{prefix}_b1"], eTh, ROWS2, ACTF.Relu, z1)
        out = [pool.tile([128, ROWS2], TRUNK, tag=f"{prefix}T_{i}", name=f"{prefix}T_{i}")
               for i in range(2)]
        mm_apply(W[f"{prefix}_W2"], W[f"{prefix}_b2"], z1, ROWS2, ACTF.Relu, out)
        return out

    # dist first so the att2 elementwise can start while the PE continues
    # with the mul projection (att1 is only needed at b0's first fold_sum)
    qscope = ctx.enter_context(ExitStack())
    qpool = qscope.enter_context(tc.tile_pool(name="qpool", bufs=1))
    qT = proj("dist", qpool)
    dbg_dump("dbg_eTh0", eTh[0][:, :])
    dbg_dump("dbg_qT0", qT[0][:, :])

    # fp16 views of q for the att2 elementwise: q1p = q1+1, q2b = q2.
    # The unused partition rows 72:128 of the low chunk are zeroed so the
    # subtract can run over all 128 partitions without stale data.
    q1p, q2b = [], []
    for ki in range(2):
        kc = CH_P[ki][1]
        tp = persist.tile([128, ROWS], H16, tag=f"q1p_{ki}", name=f"q1p_{ki}")
        t2 = persist.tile([128, ROWS], H16, tag=f"q2b_{ki}", name=f"q2b_{ki}")
        if kc < 128:
            nc.vector.memset(tp[64:128, :], 0.0)
            nc.vector.memset(t2[64:128, :], 0.0)
        nc.vector.tensor_scalar_add(out=tp[:kc, :], in0=qT[ki][:kc, :ROWS], scalar1=1.0)
        nc.vector.tensor_scalar_add(out=t2[:kc, :], in0=qT[ki][:kc, ROWS:], scalar1=0.0)
        q1p.append(tp)
        q2b.append(t2)
    qscope.close()  # frees qT

    pT = proj("mul", persist)

    # ---------------- att1 into sim4 PSUM (simT layout [j, i] per b) --------
    # start=True lazily marks the WHOLE bank pending-zero, so only the very
    # first matmul touching sim4 carries it; every later first-touch of a byte
    # overwrites, and overlapping writes accumulate. The att2 sums then
    # accumulate on top with start=False.
    sim4 = pp_sim.tile([128, 512], F32, tag="sim4", name="sim4")
    for b in range(BL):
        for ki, (ko, kc) in enumerate(CH_P):
            nc.tensor.matmul(
                out=sim4[:, b * S:(b + 1) * S],
                lhsT=pT[ki][:kc, ROWS + b * S:ROWS + (b + 1) * S],
                rhs=pT[ki][:kc, b * S:(b + 1) * S],
                start=(b == 0 and ki == 0), stop=False, skip_group_check=True,
            )

    # ---------------- att2: dist attention ----------------
    # Each j-block covers ALL 4 batch items: u layout [p, (b, j, i)]. The
    # subtracts run as small per-(b, chunk) instructions (good pipelining);
    # the rest of the elementwise runs whole-tile; the partition-sum matmuls
    # take a b-strided 512-wide rhs per j and write all of sim4's rows.
    half = JB * 512  # hi/lo chunk size in u (b, j, i)

    def att2_block(jb, type_a):
        u = upool.tile([128, 2 * half], H16, tag="u", name="u")
        for ki, off in ((0, 0), (1, half)):
            for b in range(BL):
                uo = u[:128, off + b * JB * S:off + (b + 1) * JB * S].rearrange(
                    "p (j i) -> p j i", j=JB)
                q1s = q1p[ki][:128, b * S:(b + 1) * S]
                in0 = bass.AP(tensor=q1s.tensor, offset=q1s.offset,
                              ap=[q1s.ap[0], [0, JB], q1s.ap[1]])
                q2s = q2b[ki][:128, b * S + jb * JB:b * S + (jb + 1) * JB]
                in1 = bass.AP(tensor=q2s.tensor, offset=q2s.offset,
                              ap=[q2s.ap[0], q2s.ap[1], [0, S]])
                nc.vector.tensor_tensor(out=uo, in0=in0, in1=in1,
                                        op=ALU.subtract)
        if type_a:
            # v = 2-u = 1-x ; u = max(u,v) = 1+|x| ; u = 1/u
            v = vpool.tile([128, 2 * half], H16, tag="v", name="v")
            nc.vector.tensor_scalar(out=v[:, :], in0=u[:, :], scalar1=-1.0,
                                    scalar2=2.0, op0=ALU.mult, op1=ALU.add)
            nc.vector.tensor_tensor(out=u[:, :], in0=u[:, :], in1=v[:, :],
                                    op=ALU.max)
            act_recip(nc, u[:, :], u[:, :], bias=0.0)
        else:
            # u = |u-1| = |x| ; u = 1/(1+u)
            nc.scalar.activation(out=u[:, :], in_=u[:, :], func=ACTF.Abs,
                                 bias=neg1[:, :])
            act_recip(nc, u[:, :], u[:, :], bias=1.0)
        return u

    def att2_fold_sum(jb, u):
        # fold the p=128..200 chunk onto the first 72 rows of the hi chunk
        # (DVE only: gpsimd takes ~27us per fold and gates the pipeline)
        nc.vector.tensor_tensor(out=u[:72, :half], in0=u[:72, :half],
                                in1=u[:72, half:], op=ALU.add)
        for jj in range(JB):
            j = jb * JB + jj
            g, rr = j // 32, j % 32
            rbase = u[:128, jj * S:jj * S + S]
            rhs = bass.AP(tensor=rbase.tensor, offset=rbase.offset,
                          ap=[rbase.ap[0], [JB * S, BL], [1, S]])
            nc.tensor.matmul(
                out=sim4[32 * g:32 * g + 32, :],
                lhsT=zbuf[:128, 32 - rr:64 - rr],
                rhs=rhs,
                start=False, stop=(rr == 31), skip_group_check=True,
                tile_position=(0, 32 * g),
            )

    prev = None
    for jb in range(NBLK):
        type_a = (jb * A_NUM) % A_DEN < A_NUM
        u = att2_block(jb, type_a)
        if prev is not None:
            att2_fold_sum(*prev)
        prev = (jb, u)
    att2_fold_sum(*prev)
    dbg_dump("dbg_sim4", sim4[:, :])

    # ---------------- softmax + beta/alpha + compare part 1 ----------------
    def softmax_p(src_psum):
        """softmax over rows of src [128,128]; returns transposed probs bf16."""
        mx = small.tile([128, 1], F32, tag="sm_mx", name="sm_mx")
        nc.vector.tensor_reduce(out=mx[:, :], in_=src_psum, axis=AX.X,
                                op=ALU.max, negate=True)
        esb = small.tile([128, S], H16, tag="sm_e", name="sm_e")
        zs = small.tile([128, 1], F32, tag="sm_z", name="sm_z")
        nc.scalar.activation(out=esb[:, :], in_=src_psum, func=ACTF.Exp,
                             bias=mx[:, :], scale=1.0, accum_out=zs[:, :])
        rz = small.tile([128, 1], F32, tag="sm_rz", name="sm_rz")
        nc.vector.reciprocal(out=rz[:, :], in_=zs[:, :])
        pr = small.tile([128, S], H16, tag="sm_p", name="sm_p")
        nc.vector.tensor_scalar(out=pr[:, :], in0=esb[:, :], scalar1=rz[:, :],
                                scalar2=None, op0=ALU.mult)
        pt_ps = pp_tr.tile([128, 512], H16, tag="trpackb", name="trpackb")
        nc.tensor.transpose(out=pt_ps[:S, :S], in_=pr[:, :], identity=identb[:, :])
        pt = small.tile([128, S], H16, tag="sm_pt", name="sm_pt")
        nc.scalar.activation(out=pt[:, :], in_=pt_ps[:S, :S], func=ACTF.Copy)
        return pt

    # betaT trunk tiles per side: [kc, 512] bf16, col = b*128 + token
    betaT = {s: [persist.tile([128, 512], H16, tag=f"betaT{s}_{i}", name=f"betaT{s}_{i}")
                 for i in range(3)] for s in ("1", "2")}

    for b in range(BL):
        bs4 = sim4[:, b * S:(b + 1) * S]
        ptA = softmax_p(bs4)  # alpha probs^T [i, j]
        simT_sb = small.tile([128, S], F32, tag="simT_sb", name="simT_sb")
        nc.scalar.activation(out=simT_sb[:, :], in_=bs4, func=ACTF.Copy)
        sim_ps = pp_sm.tile([128, S], F32, tag="btps", name="simtr")
        nc.tensor.transpose(out=sim_ps[:S, :S], in_=simT_sb[:, :],
                            identity=identf[:, :])
        ptB = softmax_p(sim_ps[:S, :S])  # beta probs^T [j, i]

        for side, pt, eln in (("1", ptB, "2"), ("2", ptA, "1")):
            for ki, (ko, kc) in enumerate(CH_D):
                bt_ps = pp_sm.tile([128, S], F32, tag="btps", name="btps")
                nc.tensor.matmul(
                    out=bt_ps[:kc, :], lhsT=ehw_n[(eln, b)][:, ko:ko + kc],
                    rhs=pt[:, :], start=True, stop=True,
                )
                nc.scalar.activation(
                    out=betaT[side][ki][:kc, b * S:(b + 1) * S],
                    in_=bt_ps[:kc, :], func=ACTF.Copy)

    dbg_dump("dbg_betaT0", betaT["1"][0][:, :])

    # cat + compare matmul, per side over 512-col trunk halves. The cat
    # chunks (e-b, e*b) are computed on the fly right before their two
    # accumulating matmuls, so only 2 transient tiles are alive at a time.
    cmp1 = [persist.tile([128, ROWS2], H16, tag=f"cmp1_{i}", name=f"cmp1_{i}")
            for i in range(2)]
    for side in ("1", "2"):
        so = (0 if side == "1" else ROWS)
        ps2 = [pp_mm.tile([128, 512], F32, tag="mmout", name=f"cmp1ps{mi}")
               for mi in range(2)]
        for sel in range(4):  # e, beta, e-beta, e*beta
            for ki, (ko, kc) in enumerate(CH_D):
                e_sl = eTh[ki][:kc, so:so + ROWS]
                b_sl = betaT[side][ki][:kc, :]
                if sel == 1:
                    rhs = b_sl
                else:
                    cat = small.tile([128, 512], H16, tag="cat", name="cat")
                    if sel == 0:
                        nc.vector.tensor_scalar_add(out=cat[:kc, :], in0=e_sl,
                                                    scalar1=0.0)
                    else:
                        nc.vector.tensor_tensor(
                            out=cat[:kc, :], in0=e_sl, in1=b_sl,
                            op=(ALU.subtract if sel == 2 else ALU.mult))
                    rhs = cat[:kc, :]
                idx = sel * 3 + ki
                for mi, (mo, mc) in enumerate(CH_P):
                    nc.tensor.matmul(
                        out=ps2[mi][:mc, :],
                        lhsT=W["cmp_W1"][idx][:, mo:mo + mc],
                        rhs=rhs,
                        start=(idx == 0), stop=(idx == 11),
                        skip_group_check=True,
                    )
        for mi, (mo, mc) in enumerate(CH_P):
            nc.scalar.activation(
                out=cmp1[mi][:mc, so:so + ROWS], in_=ps2[mi][:mc, :],
                func=ACTF.Relu,
                bias=W["cmp_b1"][mi][:mc, :], scale=1.0,
            )

    # ---------------- compare part 2 + compare highway (trunk) --------------
    v0 = [work.tile([128, ROWS2], H16, tag=f"v0_{i}", name=f"v0_{i}") for i in range(2)]
    mm_apply(W["cmp_W2"], W["cmp_b2"], cmp1, ROWS2, ACTF.Relu, v0)
    v1 = [work.tile([128, ROWS2], H16, tag=f"v1_{i}", name=f"v1_{i}") for i in range(2)]
    highway(v0, W["chw1_Wh"], W["chw1_bh"], W["chw1_Wt"], W["chw1_bt"], P, v1)
    vT = [persist.tile([128, ROWS2], H16, tag=f"vT_{i}", name=f"vT_{i}")
          for i in range(2)]
    highway(v1, W["chw2_Wh"], W["chw2_bh"], W["chw2_Wt"], W["chw2_bt"], P, vT)
    dbg_dump("dbg_vT0", vT[0][:, :])

    # ---------------- aggregate (fp32) ----------------
    # stats[sect][ki]: [kc, BL]; sections: v1.max, v2.max, v1.sum, v2.sum
    stats = []
    for sect, (side, op) in enumerate(
            (("1", ALU.max), ("2", ALU.max), ("1", ALU.add), ("2", ALU.add))):
        so = (0 if side == "1" else ROWS)
        st = [persist.tile([128, BL], F32, tag=f"st{sect}_{i}", name=f"st{sect}_{i}")
              for i in range(2)]
        for ki, (ko, kc) in enumerate(CH_P):
            seg = vT[ki][:kc, so:so + ROWS].rearrange("p (b t) -> p b t", b=BL)
            nc.vector.tensor_reduce(
                out=st[ki][:kc, :BL], in_=seg, axis=AX.X, op=op,
            )
        stats.append(st)

    agg_rhs = [stats[s][ki] for s in range(4) for ki in range(2)]
    y1 = [persist.tile([128, BL], F32, tag=f"y1_{i}", name=f"y1_{i}") for i in range(2)]
    mm_apply(W["agg_W1"], W["agg_b1"], agg_rhs, BL, ACTF.Relu, y1)
    y2 = [persist.tile([128, BL], F32, tag=f"y2_{i}", name=f"y2_{i}") for i in range(2)]
    mm_apply(W["agg_W2"], W["agg_b2"], y1, BL, ACTF.Relu, y2)

    yt_ps = pp_sm.tile([128, S], F32, tag="btps", name="btps")
    for ki, (ko, kc) in enumerate(CH_P):
        nc.tensor.matmul(
            out=yt_ps[:C, :BL], lhsT=W["out_W"][ki][:kc, :],
            rhs=y2[ki][:kc, :], start=(ki == 0), stop=(ki == 1),
        )
    yt_sb = persist.tile([C, BL], F32, tag="yt_sb", name="yt_sb")
    nc.scalar.activation(out=yt_sb[:, :], in_=yt_ps[:C, :BL], func=ACTF.Identity,
                         bias=W["out_b"][0][:C, :], scale=1.0)
    nc.sync.dma_start(out=io["yt"][:, :], in_=yt_sb[:, :])


_NC_CACHE = {}


def _get_nc():
    if "nc" not in _NC_CACHE:
        _NC_CACHE["nc"] = build_nc()
    return _NC_CACHE["nc"]


def make_in_maps(inputs):
    """Shard full inputs into 8 per-core input maps."""
    x1 = np.ascontiguousarray(np.asarray(inputs["x1"]).astype(np.int32))
    x2 = np.ascontiguousarray(np.asarray(inputs["x2"]).astype(np.int32))
    shared = {}
    for n in WEIGHT_NAMES + ["emb"]:
        shared[n] = np.ascontiguousarray(np.asarray(inputs[n]).astype(np.float32))
    in_maps = []
    for c in range(NCORES):
        m = dict(shared)
        m["x1"] = x1[c * BL:(c + 1) * BL]
        m["x2"] = x2[c * BL:(c + 1) * BL]
        in_maps.append(m)
    return in_maps


def kernel(**inputs):
    nc = _get_nc()
    in_maps = make_in_maps(inputs)
    res = run_bass_kernel_spmd(nc, in_maps, core_ids=list(range(NCORES)))
    return np.concatenate([np.asarray(r["yt"]).T for r in res.results], axis=0)


if __name__ == "__main__":
    nc = build_nc()
    print("built ok")


# revision 72
# speedup vs baseline: 1.5761x; 1.0515x over previous
"""Trainium2 Bass kernel for nn_AttentiveModel (B=32,S=128,D=300,P=200,V=30000,C=3).

Data-parallel over batch across 8 NeuronCores (4 batch items per core, all
weights replicated). Trunk compute (highways/projections/compare) runs in
float32r on the PE (1 cycle/row at free>=256, near-fp32 precision); the
dist-attention elementwise runs in bf16 split across DVE and ScalarE.

Layout: activations live transposed [features(partitions), rows(free)] with
both sides sharing one 1024-col trunk (col = side*512 + b*128 + token), so
every shared-weight matmul/elementwise runs once over both sides.

att2[b,j,i] = sum_p 1/(1+|q1[b,i,p]-q2[b,j,p]|), streamed in j-blocks:
  type-A blocks (DVE abs):  w=(q1+1)-q2 (TT), v=2-w (TS 4x),
                            s=max(w,v)=1+|x| (TT), r=1/s (ScalarE Reciprocal)
  type-B blocks (ScalarE abs): d=q1-q2 (TT), |d| (ScalarE Abs),
                            r=1/(1+|d|) (ScalarE Reciprocal bias=1)
  then a DVE fold of the p=128..200 chunk onto the first 72 rows and a
  partition-sum via PE matmuls with a sliding ones-column lhsT accumulating
  into the sim PSUM tile on top of att1.
ScalarE Reciprocal is emitted directly as InstActivation (bass's wrapper
refuses it on accuracy grounds far below this problem's 2e-2 tolerance).
"""

import sys
from contextlib import ExitStack

import numpy as np

for _p in ("/opt/trn_rl_repo",):
    if _p not in sys.path:
        sys.path.insert(0, _p)

import concourse.bass as bass
import concourse.tile as tile
from concourse.bacc import Bacc
from concourse import mybir
from concourse.bass_utils import run_bass_kernel_spmd
from concourse.masks import make_identity

F32 = mybir.dt.float32
F32R = mybir.dt.float32r
BF = mybir.dt.bfloat16
H16 = mybir.dt.float16
I32 = mybir.dt.int32
ALU = mybir.AluOpType
ACTF = mybir.ActivationFunctionType
AX = mybir.AxisListType

TRUNK = H16  # trunk compute dtype (H16 halves LDWEIGHTS traffic vs F32R)

B, S, D, P, V, C = 32, 128, 300, 200, 30000, 3
NCORES = 8
BL = B // NCORES  # 4 batch items per core
ROWS = BL * S  # 512 per side
ROWS2 = 2 * ROWS  # both sides in one trunk

CH_D = [(0, 128), (128, 128), (256, 44)]  # 300
CH_P = [(0, 128), (128, 72)]  # 200

JB = 8  # j-block size for att2 streaming (each block covers all 4 b)
NBLK = S // JB
# fraction of j-blocks whose abs runs on DVE (type A) vs ScalarE (type B)
A_NUM, A_DEN = 1, 5

WEIGHT_NAMES = [
    "hw1_Wh", "hw1_bh", "hw1_Wt", "hw1_bt",
    "hw2_Wh", "hw2_bh", "hw2_Wt", "hw2_bt",
    "mul_W1", "mul_b1", "mul_W2", "mul_b2",
    "dist_W1", "dist_b1", "dist_W2", "dist_b2",
    "cmp_W1", "cmp_b1", "cmp_W2", "cmp_b2",
    "chw1_Wh", "chw1_bh", "chw1_Wt", "chw1_bt",
    "chw2_Wh", "chw2_bh", "chw2_Wt", "chw2_bt",
    "agg_W1", "agg_b1", "agg_W2", "agg_b2",
    "out_W", "out_b",
]

# weights kept fp32 (tiny free dims in the aggregate MLP)
F32_WEIGHTS = {"agg_W1", "agg_W2", "out_W"}


def _chunks(n):
    out = []
    o = 0
    while o < n:
        c = min(128, n - o)
        out.append((o, c))
        o += c
    return out


def act_recip(nc, out, in_, bias=0.0):
    """out = 1/(in_ + bias) in one ScalarE pass (Reciprocal activation)."""
    eng = nc.scalar
    ins_ = [
        eng.lower_ap(in_),
        mybir.ImmediateValue(dtype=mybir.dt.float32, value=bias),  # bias
        mybir.ImmediateValue(dtype=mybir.dt.float32, value=1.0),  # scale
        mybir.ImmediateValue(dtype=mybir.dt.float32, value=0.0),  # alpha
    ]
    return eng.add_instruction(
        mybir.InstActivation(
            name=eng.bass.get_next_instruction_name(),
            func=ACTF.Reciprocal,
            ins=ins_,
            outs=[eng.lower_ap(out)],
        )
    )


def build_nc(debug=False):
    nc = Bacc()

    io = {}
    io["x1"] = nc.declare_dram_parameter("x1", [BL, S], I32, isOutput=False)
    io["x2"] = nc.declare_dram_parameter("x2", [BL, S], I32, isOutput=False)
    io["emb"] = nc.declare_dram_parameter("emb", [V, D], F32, isOutput=False)
    shapes = {
        "hw1_Wh": [D, D], "hw1_bh": [D], "hw1_Wt": [D, D], "hw1_bt": [D],
        "hw2_Wh": [D, D], "hw2_bh": [D], "hw2_Wt": [D, D], "hw2_bt": [D],
        "mul_W1": [D, P], "mul_b1": [P], "mul_W2": [P, P], "mul_b2": [P],
        "dist_W1": [D, P], "dist_b1": [P], "dist_W2": [P, P], "dist_b2": [P],
        "cmp_W1": [4 * D, P], "cmp_b1": [P], "cmp_W2": [P, P], "cmp_b2": [P],
        "chw1_Wh": [P, P], "chw1_bh": [P], "chw1_Wt": [P, P], "chw1_bt": [P],
        "chw2_Wh": [P, P], "chw2_bh": [P], "chw2_Wt": [P, P], "chw2_bt": [P],
        "agg_W1": [4 * P, P], "agg_b1": [P], "agg_W2": [P, P], "agg_b2": [P],
        "out_W": [P, C], "out_b": [C],
    }
    for n in WEIGHT_NAMES:
        io[n] = nc.declare_dram_parameter(n, shapes[n], F32, isOutput=False)
    io["yt"] = nc.declare_dram_parameter("yt", [C, BL], F32, isOutput=True)
    if debug:
        io["dbg_eTh0"] = nc.declare_dram_parameter("dbg_eTh0", [128, ROWS2], F32, isOutput=True)
        io["dbg_qT0"] = nc.declare_dram_parameter("dbg_qT0", [128, ROWS2], F32, isOutput=True)
        io["dbg_sim4"] = nc.declare_dram_parameter("dbg_sim4", [128, 512], F32, isOutput=True)
        io["dbg_betaT0"] = nc.declare_dram_parameter("dbg_betaT0", [128, 512], F32, isOutput=True)
        io["dbg_vT0"] = nc.declare_dram_parameter("dbg_vT0", [128, ROWS2], F32, isOutput=True)

    with ExitStack() as ctx:
        tc = ctx.enter_context(tile.TileContext(nc))
        _emit(ctx, nc, tc, io, debug=debug)
    nc.finalize()
    return nc


def _emit(ctx, nc, tc, io, debug=False):
    def dbg_dump(name, ap):
        if not debug or name not in io:
            return
        sh = io[name].shape
        src = ap[:sh[0], :sh[1]]
        if src.space == bass.MemorySpace.PSUM:
            t = small.tile([128, 512], F32, tag="dbgps", name=name)
            nc.scalar.activation(out=t[:sh[0], :sh[1]], in_=src, func=ACTF.Copy)
            src = t[:sh[0], :sh[1]]
        nc.gpsimd.dma_start(out=io[name][:, :], in_=src)

    wpool = ctx.enter_context(tc.tile_pool(name="wpool", bufs=1))
    const = ctx.enter_context(tc.tile_pool(name="const", bufs=1))
    persist = ctx.enter_context(tc.tile_pool(name="persist", bufs=1))
    work = ctx.enter_context(tc.tile_pool(name="work", bufs=1))
    # u triple-buffers on hardware; drop to 2 in debug builds to make room
    # for the debug dump staging (timing is irrelevant in CoreSim)
    upool = ctx.enter_context(tc.tile_pool(name="upool", bufs=(2 if debug else 3)))
    vpool = ctx.enter_context(tc.tile_pool(name="vpool", bufs=1))
    small = ctx.enter_context(tc.tile_pool(name="small", bufs=2))

    pp_mm = ctx.enter_context(tc.tile_pool(name="pp_mm", bufs=2, space="PSUM"))
    pp_sim = ctx.enter_context(tc.tile_pool(name="pp_sim", bufs=1, space="PSUM"))
    pp_tr = ctx.enter_context(tc.tile_pool(name="pp_tr", bufs=2, space="PSUM"))
    pp_r = ctx.enter_context(tc.tile_pool(name="pp_r", bufs=1, space="PSUM"))
    pp_sm = ctx.enter_context(tc.tile_pool(name="pp_sm", bufs=2, space="PSUM"))

    # ---------------- constants ----------------
    identf = const.tile([128, 128], F32, tag="identf", name="identf")
    make_identity(nc, identf[:, :])
    identr = const.tile([128, 128], TRUNK, tag="identr", name="identr")
    nc.vector.tensor_scalar_add(out=identr[:, :], in0=identf[:, :], scalar1=0.0)
    identb = const.tile([128, 128], H16, tag="identb", name="identb")
    nc.vector.tensor_scalar_add(out=identb[:, :], in0=identf[:, :], scalar1=0.0)

    # sliding ones-column buffer: Z[:, 32] == 1 so Z[:, 32-r:64-r] has its
    # ones in column r; Z_slice.T @ U deposits column-sums of U into row r.
    zbuf = const.tile([128, 64], H16, tag="zbuf", name="zbuf")
    nc.vector.memset(zbuf[:, :], 0.0)
    nc.vector.memset(zbuf[:, 32:33], 1.0)

    neg1 = const.tile([128, 1], F32, tag="neg1", name="neg1")
    nc.vector.memset(neg1[:, :], -1.0)

    # ---------------- weights: casting DMAs via gpsimd queue --------------
    SPECIAL_KCH = {
        "cmp_W1": [(s * D + o, c) for s in range(4) for (o, c) in CH_D],
        "agg_W1": [(s * P + o, c) for s in range(4) for (o, c) in CH_P],
    }

    def load_w(name):
        h = io[name]
        K, M = h.shape
        H16_W = {"cmp_W1", "cmp_W2", "chw1_Wh", "chw1_Wt", "chw2_Wh", "chw2_Wt"}
        dt = F32 if name in F32_WEIGHTS else (H16 if name in H16_W else TRUNK)
        tiles = []
        for i, (o, c) in enumerate(SPECIAL_KCH.get(name, _chunks(K))):
            t = wpool.tile([c, M], dt, tag=f"w_{name}_{i}", name=f"w_{name}_{i}")
            eng = nc.sync if dt == F32 else nc.gpsimd
            eng.dma_start(out=t[:, :], in_=h[o:o + c, :])
            tiles.append(t)
        return tiles

    def load_b(name):
        h = io[name]
        (M,) = h.shape
        tiles = []
        for i, (o, c) in enumerate(_chunks(M)):
            t = wpool.tile([c, 1], F32, tag=f"b_{name}_{i}", name=f"b_{name}_{i}")
            nc.sync.dma_start(out=t[:, :], in_=h[o:o + c])
            tiles.append(t)
        return tiles

    # ---------------- index DMAs + gathers (overlap weight DMAs) ----------
    pre2 = ctx.enter_context(ExitStack())
    g2pool = pre2.enter_context(tc.tile_pool(name="g2pool", bufs=1))
    pre1 = ctx.enter_context(ExitStack())
    gpool = pre1.enter_context(tc.tile_pool(name="gpool", bufs=1))
    e_n = {}
    for side, xh in (("1", io["x1"]), ("2", io["x2"])):
        for b in range(BL):
            idx = gpool.tile([128, 1], I32, tag=f"idx{side}_{b}", name=f"idx{side}_{b}")
            nc.sync.dma_start(out=idx[:, :], in_=xh[b, :])
            e = gpool.tile([128, D], H16, tag=f"e{side}_{b}", name=f"e{side}_{b}")
            nc.gpsimd.indirect_dma_start(
                out=e[:, :], out_offset=None, in_=io["emb"][:, :],
                in_offset=bass.IndirectOffsetOnAxis(ap=idx[:, :1], axis=0),
            )
            e_n[(side, b)] = e

    W = {}
    for n in WEIGHT_NAMES:
        W[n] = load_b(n) if n.endswith(("bh", "bt", "b1", "b2", "_b")) else load_w(n)

    # ---------------- helpers ----------------
    def mm_apply(w_tiles, b_tiles, rhs_tiles, n_free, func, out_tiles,
                 krange=None, mrange=None):
        """out = func(W.T @ rhs + b), transposed layout, 512-col PSUM chunks."""
        M = w_tiles[0].shape[1]
        mch = _chunks(M)
        ks = list(range(len(w_tiles))) if krange is None else krange
        m_iter = ([(i, i) for i in range(len(mch))] if mrange is None
                  else list(enumerate(mrange)))
        for oi, mi in m_iter:
            mo, mc = mch[mi]
            for fo in range(0, n_free, 512):
                fc = min(512, n_free - fo)
                ps = pp_mm.tile([128, 512], F32, tag="mmout", name="mmout")
                for idx, ki in enumerate(ks):
                    kc = w_tiles[ki].shape[0]
                    nc.tensor.matmul(
                        out=ps[:mc, :fc],
                        lhsT=w_tiles[ki][:kc, mo:mo + mc],
                        rhs=rhs_tiles[ki][:kc, fo:fo + fc],
                        start=(idx == 0),
                        stop=(idx == len(ks) - 1),
                    )
                nc.scalar.activation(
                    out=out_tiles[oi][:mc, fo:fo + fc],
                    in_=ps[:mc, :fc],
                    func=func, bias=b_tiles[mi][:mc, :], scale=1.0,
                )

    def highway(xt_tiles, wh, bh, wt, bt, feat, out_tiles):
        """out = x + t*(h-x), trunk layout, chunk-at-a-time (h reused as tmp)."""
        ch = _chunks(feat)
        for mi, (mo, mc) in enumerate(ch):
            h = work.tile([128, ROWS2], TRUNK, tag="hw_h", name="hw_h")
            t = work.tile([128, ROWS2], TRUNK, tag="hw_t", name="hw_t")
            mm_apply(wh, bh, xt_tiles, ROWS2, ACTF.Relu, [h], mrange=[mi])
            mm_apply(wt, bt, xt_tiles, ROWS2, ACTF.Sigmoid, [t], mrange=[mi])
            x_sl = xt_tiles[mi][:mc, :]
            nc.vector.tensor_tensor(out=h[:mc, :], in0=h[:mc, :], in1=x_sl,
                                    op=ALU.subtract)
            nc.vector.tensor_tensor(out=h[:mc, :], in0=h[:mc, :], in1=t[:mc, :],
                                    op=ALU.mult)
            nc.vector.tensor_tensor(out=out_tiles[mi][:mc, :], in0=h[:mc, :],
                                    in1=x_sl, op=ALU.add)

    # ---------------- embed: transpose into trunk ----------------
    # eT[ki]: [kc, 1024], col = side*512 + b*128 + token
    eT = [g2pool.tile([128, ROWS2], TRUNK, tag=f"eT_{i}", name=f"eT_{i}")
          for i in range(3)]
    for ki, (ko, kc) in enumerate(CH_D):
        for side in ("1", "2"):
            ps = pp_tr.tile([128, 512], H16, tag="trpackb", name="trpack")
            for b in range(BL):
                nc.tensor.transpose(
                    out=ps[:kc, b * S:(b + 1) * S],
                    in_=e_n[(side, b)][:, ko:ko + kc],
                    identity=identb[:128, :128],
                )
            so = (0 if side == "1" else ROWS)
            nc.scalar.activation(out=eT[ki][:kc, so:so + ROWS], in_=ps[:kc, :ROWS],
                                 func=ACTF.Copy)
    pre1.close()  # frees index + gather tiles

    # ---------------- highway stack (trunk: both sides at once) -------------
    h1 = [g2pool.tile([128, ROWS2], TRUNK, tag=f"hwy1_{i}", name=f"hwy1_{i}")
          for i in range(3)]
    highway(eT, W["hw1_Wh"], W["hw1_bh"], W["hw1_Wt"], W["hw1_bt"], D, h1)
    eTh = [persist.tile([128, ROWS2], TRUNK, tag=f"eTh_{i}", name=f"eTh_{i}")
           for i in range(3)]
    highway(h1, W["hw2_Wh"], W["hw2_bh"], W["hw2_Wt"], W["hw2_bt"], D, eTh)
    pre2.close()  # frees eT, h1

    # normal-layout post-highway embeddings (lhsT for the beta/alpha matmuls)
    ehw_n = {}
    for side in ("1", "2"):
        so = (0 if side == "1" else ROWS)
        for b in range(BL):
            ps = pp_r.tile([128, 512], TRUNK, tag="trpackr", name="trpackr")
            for ki, (ko, kc) in enumerate(CH_D):
                nc.tensor.transpose(
                    out=ps[:128, ko:ko + kc],
                    in_=eTh[ki][:kc, so + b * S:so + (b + 1) * S],
                    identity=identr[:kc, :kc],
                )
            t = persist.tile([128, D], H16, tag=f"ehwn{side}_{b}", name=f"ehwn{side}_{b}")
            nc.scalar.activation(out=t[:, :], in_=ps[:, :D], func=ACTF.Copy)
            ehw_n[(side, b)] = t

    # ---------------- projections (shared weights, trunk) ----------------
    def proj(prefix, pool):
        z1 = [work.tile([128, ROWS2], TRUNK, tag=f"z1_{i}", name=f"z1_{i}") for i in range(2)]
        mm_apply(W[f"{prefix}_W1"], W[f"{prefix}_b1"], eTh, ROWS2, ACTF.Relu, z1)
        out = [pool.tile([128, ROWS2], TRUNK, tag=f"{prefix}T_{i}", name=f"{prefix}T_{i}")
               for i in range(2)]
        mm_apply(W[f"{prefix}_W2"], W[f"{prefix}_b2"], z1, ROWS2, ACTF.Relu, out)
        return out

    # dist first so the att2 elementwise can start while the PE continues
    # with the mul projection (att1 is only needed at b0's first fold_sum)
    qscope = ctx.enter_context(ExitStack())
    qpool = qscope.enter_context(tc.tile_pool(name="qpool", bufs=1))
    qT = proj("dist", qpool)
    dbg_dump("dbg_eTh0", eTh[0][:, :])
    dbg_dump("dbg_qT0", qT[0][:, :])

    # fp16 views of q for the att2 elementwise: q1p = q1+1, q2b = q2.
    # The unused partition rows 72:128 of the low chunk are zeroed so the
    # subtract can run over all 128 partitions without stale data.
    q1p, q2b = [], []
    for ki in range(2):
        kc = CH_P[ki][1]
        tp = persist.tile([128, ROWS], H16, tag=f"q1p_{ki}", name=f"q1p_{ki}")
        t2 = persist.tile([128, ROWS], H16, tag=f"q2b_{ki}", name=f"q2b_{ki}")
        if kc < 128:
            nc.vector.memset(tp[64:128, :], 0.0)
            nc.vector.memset(t2[64:128, :], 0.0)
        nc.vector.tensor_scalar_add(out=tp[:kc, :], in0=qT[ki][:kc, :ROWS], scalar1=1.0)
        nc.vector.tensor_scalar_add(out=t2[:kc, :], in0=qT[ki][:kc, ROWS:], scalar1=0.0)
        q1p.append(tp)
        q2b.append(t2)
    qscope.close()  # frees qT

    pT = proj("mul", persist)

    # ---------------- att1 into sim4 PSUM (simT layout [j, i] per b) --------
    # start=True lazily marks the WHOLE bank pending-zero, so only the very
    # first matmul touching sim4 carries it; every later first-touch of a byte
    # overwrites, and overlapping writes accumulate. The att2 sums then
    # accumulate on top with start=False.
    sim4 = pp_sim.tile([128, 512], F32, tag="sim4", name="sim4")
    for b in range(BL):
        for ki, (ko, kc) in enumerate(CH_P):
            nc.tensor.matmul(
                out=sim4[:, b * S:(b + 1) * S],
                lhsT=pT[ki][:kc, ROWS + b * S:ROWS + (b + 1) * S],
                rhs=pT[ki][:kc, b * S:(b + 1) * S],
                start=(b == 0 and ki == 0), stop=False, skip_group_check=True,
            )

    # ---------------- att2: dist attention ----------------
    # Each j-block covers ALL 4 batch items: u layout [p, (b, j, i)]. The
    # subtracts run as small per-(b, chunk) instructions (good pipelining);
    # the rest of the elementwise runs whole-tile; the partition-sum matmuls
    # take a b-strided 512-wide rhs per j and write all of sim4's rows.
    half = JB * 512  # hi/lo chunk size in u (b, j, i)

    def att2_block(jb, type_a):
        u = upool.tile([128, 2 * half], H16, tag="u", name="u")
        for ki, off in ((0, 0), (1, half)):
            for b in range(BL):
                uo = u[:128, off + b * JB * S:off + (b + 1) * JB * S].rearrange(
                    "p (j i) -> p j i", j=JB)
                q1s = q1p[ki][:128, b * S:(b + 1) * S]
                in0 = bass.AP(tensor=q1s.tensor, offset=q1s.offset,
                              ap=[q1s.ap[0], [0, JB], q1s.ap[1]])
                q2s = q2b[ki][:128, b * S + jb * JB:b * S + (jb + 1) * JB]
                in1 = bass.AP(tensor=q2s.tensor, offset=q2s.offset,
                              ap=[q2s.ap[0], q2s.ap[1], [0, S]])
                nc.vector.tensor_tensor(out=uo, in0=in0, in1=in1,
                                        op=ALU.subtract)
        if type_a:
            # v = 2-u = 1-x ; u = max(u,v) = 1+|x| ; u = 1/u
            v = vpool.tile([128, 2 * half], H16, tag="v", name="v")
            nc.vector.tensor_scalar(out=v[:, :], in0=u[:, :], scalar1=-1.0,
                                    scalar2=2.0, op0=ALU.mult, op1=ALU.add)
            nc.vector.tensor_tensor(out=u[:, :], in0=u[:, :], in1=v[:, :],
                                    op=ALU.max)
            act_recip(nc, u[:, :], u[:, :], bias=0.0)
        else:
            # u = |u-1| = |x| ; u = 1/(1+u)
            nc.scalar.activation(out=u[:, :], in_=u[:, :], func=ACTF.Abs,
                                 bias=neg1[:, :])
            act_recip(nc, u[:, :], u[:, :], bias=1.0)
        return u

    def att2_fold_sum(jb, u):
        # fold the p=128..200 chunk onto the first 72 rows of the hi chunk
        # (DVE only: gpsimd takes ~27us per fold and gates the pipeline)
        nc.vector.tensor_tensor(out=u[:72, :half], in0=u[:72, :half],
                                in1=u[:72, half:], op=ALU.add)
        for jj in range(JB):
            j = jb * JB + jj
            g, rr = j // 32, j % 32
            rbase = u[:128, jj * S:jj * S + S]
            rhs = bass.AP(tensor=rbase.tensor, offset=rbase.offset,
                          ap=[rbase.ap[0], [JB * S, BL], [1, S]])
            nc.tensor.matmul(
                out=sim4[32 * g:32 * g + 32, :],
                lhsT=zbuf[:128, 32 - rr:64 - rr],
                rhs=rhs,
                start=False, stop=(rr == 31), skip_group_check=True,
                tile_position=(0, 32 * g),
            )

    prev = None
    for jb in range(NBLK):
        type_a = (jb * A_NUM) % A_DEN < A_NUM
        u = att2_block(jb, type_a)
        if prev is not None:
            att2_fold_sum(*prev)
        prev = (jb, u)
    att2_fold_sum(*prev)
    dbg_dump("dbg_sim4", sim4[:, :])

    # ---------------- softmax + beta/alpha + compare part 1 ----------------
    def softmax_p(src_psum):
        """softmax over rows of src [128,128]; returns transposed probs bf16."""
        mx = small.tile([128, 1], F32, tag="sm_mx", name="sm_mx")
        nc.vector.tensor_reduce(out=mx[:, :], in_=src_psum, axis=AX.X,
                                op=ALU.max, negate=True)
        esb = small.tile([128, S], H16, tag="sm_e", name="sm_e")
        zs = small.tile([128, 1], F32, tag="sm_z", name="sm_z")
        nc.scalar.activation(out=esb[:, :], in_=src_psum, func=ACTF.Exp,
                             bias=mx[:, :], scale=1.0, accum_out=zs[:, :])
        rz = small.tile([128, 1], F32, tag="sm_rz", name="sm_rz")
        nc.vector.reciprocal(out=rz[:, :], in_=zs[:, :])
        pr = small.tile([128, S], H16, tag="sm_p", name="sm_p")
        nc.vector.tensor_scalar(out=pr[:, :], in0=esb[:, :], scalar1=rz[:, :],
                                scalar2=None, op0=ALU.mult)
        pt_ps = pp_tr.tile([128, 512], H16, tag="trpackb", name="trpackb")
        nc.tensor.transpose(out=pt_ps[:S, :S], in_=pr[:, :], identity=identb[:, :])
        pt = small.tile([128, S], H16, tag="sm_pt", name="sm_pt")
        nc.scalar.activation(out=pt[:, :], in_=pt_ps[:S, :S], func=ACTF.Copy)
        return pt

    # betaT trunk tiles per side: [kc, 512] bf16, col = b*128 + token
    betaT = {s: [persist.tile([128, 512], H16, tag=f"betaT{s}_{i}", name=f"betaT{s}_{i}")
                 for i in range(3)] for s in ("1", "2")}

    for b in range(BL):
        bs4 = sim4[:, b * S:(b + 1) * S]
        ptA = softmax_p(bs4)  # alpha probs^T [i, j]
        simT_sb = small.tile([128, S], F32, tag="simT_sb", name="simT_sb")
        nc.scalar.activation(out=simT_sb[:, :], in_=bs4, func=ACTF.Copy)
        sim_ps = pp_sm.tile([128, S], F32, tag="btps", name="simtr")
        nc.tensor.transpose(out=sim_ps[:S, :S], in_=simT_sb[:, :],
                            identity=identf[:, :])
        ptB = softmax_p(sim_ps[:S, :S])  # beta probs^T [j, i]

        for side, pt, eln in (("1", ptB, "2"), ("2", ptA, "1")):
            for ki, (ko, kc) in enumerate(CH_D):
                bt_ps = pp_sm.tile([128, S], F32, tag="btps", name="btps")
                nc.tensor.matmul(
                    out=bt_ps[:kc, :], lhsT=ehw_n[(eln, b)][:, ko:ko + kc],
                    rhs=pt[:, :], start=True, stop=True,
                )
                nc.scalar.activation(
                    out=betaT[side][ki][:kc, b * S:(b + 1) * S],
                    in_=bt_ps[:kc, :], func=ACTF.Copy)

    dbg_dump("dbg_betaT0", betaT["1"][0][:, :])

    # cat + compare matmul, per side over 512-col trunk halves. The cat
    # chunks (e-b, e*b) are computed on the fly right before their two
    # accumulating matmuls, so only 2 transient tiles are alive at a time.
    cmp1 = [persist.tile([128, ROWS2], H16, tag=f"cmp1_{i}", name=f"cmp1_{i}")
            for i in range(2)]
    for side in ("1", "2"):
        so = (0 if side == "1" else ROWS)
        ps2 = [pp_mm.tile([128, 512], F32, tag="mmout", name=f"cmp1ps{mi}")
               for mi in range(2)]
        for sel in range(4):  # e, beta, e-beta, e*beta
            for ki, (ko, kc) in enumerate(CH_D):
                e_sl = eTh[ki][:kc, so:so + ROWS]
                b_sl = betaT[side][ki][:kc, :]
                if sel == 1:
                    rhs = b_sl
                else:
                    cat = small.tile([128, 512], H16, tag="cat", name="cat")
                    if sel == 0:
                        nc.vector.tensor_scalar_add(out=cat[:kc, :], in0=e_sl,
                                                    scalar1=0.0)
                    else:
                        nc.vector.tensor_tensor(
                            out=cat[:kc, :], in0=e_sl, in1=b_sl,
                            op=(ALU.subtract if sel == 2 else ALU.mult))
                    rhs = cat[:kc, :]
                idx = sel * 3 + ki
                for mi, (mo, mc) in enumerate(CH_P):
                    nc.tensor.matmul(
                        out=ps2[mi][:mc, :],
                        lhsT=W["cmp_W1"][idx][:, mo:mo + mc],
                        rhs=rhs,
                        start=(idx == 0), stop=(idx == 11),
                        skip_group_check=True,
                    )
        for mi, (mo, mc) in enumerate(CH_P):
            nc.scalar.activation(
                out=cmp1[mi][:mc, so:so + ROWS], in_=ps2[mi][:mc, :],
                func=ACTF.Relu,
                bias=W["cmp_b1"][mi][:mc, :], scale=1.0,
            )

    # ---------------- compare part 2 + compare highway (trunk) --------------
    v0 = [work.tile([128, ROWS2], H16, tag=f"v0_{i}", name=f"v0_{i}") for i in range(2)]
    mm_apply(W["cmp_W2"], W["cmp_b2"], cmp1, ROWS2, ACTF.Relu, v0)
    v1 = [work.tile([128, ROWS2], H16, tag=f"v1_{i}", name=f"v1_{i}") for i in range(2)]
    highway(v0, W["chw1_Wh"], W["chw1_bh"], W["chw1_Wt"], W["chw1_bt"], P, v1)
    vT = [persist.tile([128, ROWS2], H16, tag=f"vT_{i}", name=f"vT_{i}")
          for i in range(2)]
    highway(v1, W["chw2_Wh"], W["chw2_bh"], W["chw2_Wt"], W["chw2_bt"], P, vT)
    dbg_dump("dbg_vT0", vT[0][:, :])

    # ---------------- aggregate (fp32) ----------------
    # stats[sect][ki]: [kc, BL]; sections: v1.max, v2.max, v1.sum, v2.sum
    stats = []
    for sect, (side, op) in enumerate(
            (("1", ALU.max), ("2", ALU.max), ("1", ALU.add), ("2", ALU.add))):
        so = (0 if side == "1" else ROWS)
        st = [persist.tile([128, BL], F32, tag=f"st{sect}_{i}", name=f"st{sect}_{i}")
              for i in range(2)]
        for ki, (ko, kc) in enumerate(CH_P):
            seg = vT[ki][:kc, so:so + ROWS].rearrange("p (b t) -> p b t", b=BL)
            nc.vector.tensor_reduce(
                out=st[ki][:kc, :BL], in_=seg, axis=AX.X, op=op,
            )
        stats.append(st)

    agg_rhs = [stats[s][ki] for s in range(4) for ki in range(2)]
    y1 = [persist.tile([128, BL], F32, tag=f"y1_{i}", name=f"y1_{i}") for i in range(2)]
    mm_apply(W["agg_W1"], W["agg_b1"], agg_rhs, BL, ACTF.Relu, y1)
    y2 = [persist.tile([128, BL], F32, tag=f"y2_{i}", name=f"y2_{i}") for i in range(2)]
    mm_apply(W["agg_W2"], W["agg_b2"], y1, BL, ACTF.Relu, y2)

    yt_ps = pp_sm.tile([128, S], F32, tag="btps", name="btps")
    for ki, (ko, kc) in enumerate(CH_P):
        nc.tensor.matmul(
            out=yt_ps[:C, :BL], lhsT=W["out_W"][ki][:kc, :],
            rhs=y2[ki][:kc, :], start=(ki == 0), stop=(ki == 1),
        )
    yt_sb = persist.tile([C, BL], F32, tag="yt_sb", name="yt_sb")
    nc.scalar.activation(out=yt_sb[:, :], in_=yt_ps[:C, :BL], func=ACTF.Identity,
                         bias=W["out_b"][0][:C, :], scale=1.0)
    nc.sync.dma_start(out=io["yt"][:, :], in_=yt_sb[:, :])


_NC_CACHE = {}


def _get_nc():
    if "nc" not in _NC_CACHE:
        _NC_CACHE["nc"] = build_nc()
    return _NC_CACHE["nc"]


def make_in_maps(inputs):
    """Shard full inputs into 8 per-core input maps."""
    x1 = np.ascontiguousarray(np.asarray(inputs["x1"]).astype(np.int32))
    x2 = np.ascontiguousarray(np.asarray(inputs["x2"]).astype(np.int32))
    shared = {}
    for n in WEIGHT_NAMES + ["emb"]:
        shared[n] = np.ascontiguousarray(np.asarray(inputs[n]).astype(np.float32))
    in_maps = []
    for c in range(NCORES):
        m = dict(shared)
        m["x1"] = x1[c * BL:(c + 1) * BL]
        m["x2"] = x2[c * BL:(c + 1) * BL]
        in_maps.append(m)
    return in_maps


def kernel(**inputs):
    nc = _get_nc()
    in_maps = make_in_maps(inputs)
    res = run_bass_kernel_spmd(nc, in_maps, core_ids=list(range(NCORES)))
    return np.concatenate([np.asarray(r["yt"]).T for r in res.results], axis=0)


if __name__ == "__main__":
    nc = build_nc()
    print("built ok")


# revision 74
# speedup vs baseline: 1.5963x; 1.0128x over previous
"""Trainium2 Bass kernel for nn_AttentiveModel (B=32,S=128,D=300,P=200,V=30000,C=3).

Data-parallel over batch across 8 NeuronCores (4 batch items per core, all
weights replicated). Trunk compute (highways/projections/compare) runs in
float32r on the PE (1 cycle/row at free>=256, near-fp32 precision); the
dist-attention elementwise runs in bf16 split across DVE and ScalarE.

Layout: activations live transposed [features(partitions), rows(free)] with
both sides sharing one 1024-col trunk (col = side*512 + b*128 + token), so
every shared-weight matmul/elementwise runs once over both sides.

att2[b,j,i] = sum_p 1/(1+|q1[b,i,p]-q2[b,j,p]|), streamed in j-blocks:
  type-A blocks (DVE abs):  w=(q1+1)-q2 (TT), v=2-w (TS 4x),
                            s=max(w,v)=1+|x| (TT), r=1/s (ScalarE Reciprocal)
  type-B blocks (ScalarE abs): d=q1-q2 (TT), |d| (ScalarE Abs),
                            r=1/(1+|d|) (ScalarE Reciprocal bias=1)
  then a DVE fold of the p=128..200 chunk onto the first 72 rows and a
  partition-sum via PE matmuls with a sliding ones-column lhsT accumulating
  into the sim PSUM tile on top of att1.
ScalarE Reciprocal is emitted directly as InstActivation (bass's wrapper
refuses it on accuracy grounds far below this problem's 2e-2 tolerance).
"""

import sys
from contextlib import ExitStack

import numpy as np

for _p in ("/opt/trn_rl_repo",):
    if _p not in sys.path:
        sys.path.insert(0, _p)

import concourse.bass as bass
import concourse.tile as tile
from concourse.bacc import Bacc
from concourse import mybir
from concourse.bass_utils import run_bass_kernel_spmd
from concourse.masks import make_identity

F32 = mybir.dt.float32
F32R = mybir.dt.float32r
BF = mybir.dt.bfloat16
H16 = mybir.dt.float16
I32 = mybir.dt.int32
ALU = mybir.AluOpType
ACTF = mybir.ActivationFunctionType
AX = mybir.AxisListType

TRUNK = H16  # trunk compute dtype (H16 halves LDWEIGHTS traffic vs F32R)

B, S, D, P, V, C = 32, 128, 300, 200, 30000, 3
NCORES = 8
BL = B // NCORES  # 4 batch items per core
ROWS = BL * S  # 512 per side
ROWS2 = 2 * ROWS  # both sides in one trunk

CH_D = [(0, 128), (128, 128), (256, 44)]  # 300
CH_P = [(0, 128), (128, 72)]  # 200

JB = 8  # j-block size for att2 streaming (each block covers all 4 b)
NBLK = S // JB
# fraction of j-blocks whose abs runs on DVE (type A) vs ScalarE (type B)
A_NUM, A_DEN = 1, 4

WEIGHT_NAMES = [
    "hw1_Wh", "hw1_bh", "hw1_Wt", "hw1_bt",
    "hw2_Wh", "hw2_bh", "hw2_Wt", "hw2_bt",
    "mul_W1", "mul_b1", "mul_W2", "mul_b2",
    "dist_W1", "dist_b1", "dist_W2", "dist_b2",
    "cmp_W1", "cmp_b1", "cmp_W2", "cmp_b2",
    "chw1_Wh", "chw1_bh", "chw1_Wt", "chw1_bt",
    "chw2_Wh", "chw2_bh", "chw2_Wt", "chw2_bt",
    "agg_W1", "agg_b1", "agg_W2", "agg_b2",
    "out_W", "out_b",
]

# weights kept fp32 (tiny free dims in the aggregate MLP)
F32_WEIGHTS = {"agg_W1", "agg_W2", "out_W"}


def _chunks(n):
    out = []
    o = 0
    while o < n:
        c = min(128, n - o)
        out.append((o, c))
        o += c
    return out


def act_recip(nc, out, in_, bias=0.0):
    """out = 1/(in_ + bias) in one ScalarE pass (Reciprocal activation)."""
    eng = nc.scalar
    ins_ = [
        eng.lower_ap(in_),
        mybir.ImmediateValue(dtype=mybir.dt.float32, value=bias),  # bias
        mybir.ImmediateValue(dtype=mybir.dt.float32, value=1.0),  # scale
        mybir.ImmediateValue(dtype=mybir.dt.float32, value=0.0),  # alpha
    ]
    return eng.add_instruction(
        mybir.InstActivation(
            name=eng.bass.get_next_instruction_name(),
            func=ACTF.Reciprocal,
            ins=ins_,
            outs=[eng.lower_ap(out)],
        )
    )


def build_nc(debug=False):
    nc = Bacc()

    io = {}
    io["x1"] = nc.declare_dram_parameter("x1", [BL, S], I32, isOutput=False)
    io["x2"] = nc.declare_dram_parameter("x2", [BL, S], I32, isOutput=False)
    io["emb"] = nc.declare_dram_parameter("emb", [V, D], F32, isOutput=False)
    shapes = {
        "hw1_Wh": [D, D], "hw1_bh": [D], "hw1_Wt": [D, D], "hw1_bt": [D],
        "hw2_Wh": [D, D], "hw2_bh": [D], "hw2_Wt": [D, D], "hw2_bt": [D],
        "mul_W1": [D, P], "mul_b1": [P], "mul_W2": [P, P], "mul_b2": [P],
        "dist_W1": [D, P], "dist_b1": [P], "dist_W2": [P, P], "dist_b2": [P],
        "cmp_W1": [4 * D, P], "cmp_b1": [P], "cmp_W2": [P, P], "cmp_b2": [P],
        "chw1_Wh": [P, P], "chw1_bh": [P], "chw1_Wt": [P, P], "chw1_bt": [P],
        "chw2_Wh": [P, P], "chw2_bh": [P], "chw2_Wt": [P, P], "chw2_bt": [P],
        "agg_W1": [4 * P, P], "agg_b1": [P], "agg_W2": [P, P], "agg_b2": [P],
        "out_W": [P, C], "out_b": [C],
    }
    for n in WEIGHT_NAMES:
        io[n] = nc.declare_dram_parameter(n, shapes[n], F32, isOutput=False)
    io["yt"] = nc.declare_dram_parameter("yt", [C, BL], F32, isOutput=True)
    if debug:
        io["dbg_eTh0"] = nc.declare_dram_parameter("dbg_eTh0", [128, ROWS2], F32, isOutput=True)
        io["dbg_qT0"] = nc.declare_dram_parameter("dbg_qT0", [128, ROWS2], F32, isOutput=True)
        io["dbg_sim4"] = nc.declare_dram_parameter("dbg_sim4", [128, 512], F32, isOutput=True)
        io["dbg_betaT0"] = nc.declare_dram_parameter("dbg_betaT0", [128, 512], F32, isOutput=True)
        io["dbg_vT0"] = nc.declare_dram_parameter("dbg_vT0", [128, ROWS2], F32, isOutput=True)

    with ExitStack() as ctx:
        tc = ctx.enter_context(tile.TileContext(nc))
        _emit(ctx, nc, tc, io, debug=debug)
    nc.finalize()
    return nc


def _emit(ctx, nc, tc, io, debug=False):
    def dbg_dump(name, ap):
        if not debug or name not in io:
            return
        sh = io[name].shape
        src = ap[:sh[0], :sh[1]]
        if src.space == bass.MemorySpace.PSUM:
            t = small.tile([128, 512], F32, tag="dbgps", name=name)
            nc.scalar.activation(out=t[:sh[0], :sh[1]], in_=src, func=ACTF.Copy)
            src = t[:sh[0], :sh[1]]
        nc.gpsimd.dma_start(out=io[name][:, :], in_=src)

    wpool = ctx.enter_context(tc.tile_pool(name="wpool", bufs=1))
    const = ctx.enter_context(tc.tile_pool(name="const", bufs=1))
    persist = ctx.enter_context(tc.tile_pool(name="persist", bufs=1))
    work = ctx.enter_context(tc.tile_pool(name="work", bufs=1))
    # u triple-buffers on hardware; drop to 2 in debug builds to make room
    # for the debug dump staging (timing is irrelevant in CoreSim)
    upool = ctx.enter_context(tc.tile_pool(name="upool", bufs=(2 if debug else 4)))
    vpool = ctx.enter_context(tc.tile_pool(name="vpool", bufs=1))
    small = ctx.enter_context(tc.tile_pool(name="small", bufs=2))

    pp_mm = ctx.enter_context(tc.tile_pool(name="pp_mm", bufs=2, space="PSUM"))
    pp_sim = ctx.enter_context(tc.tile_pool(name="pp_sim", bufs=1, space="PSUM"))
    pp_tr = ctx.enter_context(tc.tile_pool(name="pp_tr", bufs=2, space="PSUM"))
    pp_r = ctx.enter_context(tc.tile_pool(name="pp_r", bufs=1, space="PSUM"))
    pp_sm = ctx.enter_context(tc.tile_pool(name="pp_sm", bufs=2, space="PSUM"))

    # ---------------- constants ----------------
    identf = const.tile([128, 128], F32, tag="identf", name="identf")
    make_identity(nc, identf[:, :])
    identr = const.tile([128, 128], TRUNK, tag="identr", name="identr")
    nc.vector.tensor_scalar_add(out=identr[:, :], in0=identf[:, :], scalar1=0.0)
    identb = const.tile([128, 128], H16, tag="identb", name="identb")
    nc.vector.tensor_scalar_add(out=identb[:, :], in0=identf[:, :], scalar1=0.0)

    # sliding ones-column buffer: Z[:, 32] == 1 so Z[:, 32-r:64-r] has its
    # ones in column r; Z_slice.T @ U deposits column-sums of U into row r.
    zbuf = const.tile([128, 64], H16, tag="zbuf", name="zbuf")
    nc.vector.memset(zbuf[:, :], 0.0)
    nc.vector.memset(zbuf[:, 32:33], 1.0)

    neg1 = const.tile([128, 1], F32, tag="neg1", name="neg1")
    nc.vector.memset(neg1[:, :], -1.0)

    # ---------------- weights: casting DMAs via gpsimd queue --------------
    SPECIAL_KCH = {
        "cmp_W1": [(s * D + o, c) for s in range(4) for (o, c) in CH_D],
        "agg_W1": [(s * P + o, c) for s in range(4) for (o, c) in CH_P],
    }

    def load_w(name):
        h = io[name]
        K, M = h.shape
        H16_W = {"cmp_W1", "cmp_W2", "chw1_Wh", "chw1_Wt", "chw2_Wh", "chw2_Wt"}
        dt = F32 if name in F32_WEIGHTS else (H16 if name in H16_W else TRUNK)
        tiles = []
        for i, (o, c) in enumerate(SPECIAL_KCH.get(name, _chunks(K))):
            t = wpool.tile([c, M], dt, tag=f"w_{name}_{i}", name=f"w_{name}_{i}")
            eng = nc.sync if dt == F32 else nc.gpsimd
            eng.dma_start(out=t[:, :], in_=h[o:o + c, :])
            tiles.append(t)
        return tiles

    def load_b(name):
        h = io[name]
        (M,) = h.shape
        tiles = []
        for i, (o, c) in enumerate(_chunks(M)):
            t = wpool.tile([c, 1], F32, tag=f"b_{name}_{i}", name=f"b_{name}_{i}")
            nc.sync.dma_start(out=t[:, :], in_=h[o:o + c])
            tiles.append(t)
        return tiles

    # ---------------- index DMAs + gathers (overlap weight DMAs) ----------
    pre2 = ctx.enter_context(ExitStack())
    g2pool = pre2.enter_context(tc.tile_pool(name="g2pool", bufs=1))
    pre1 = ctx.enter_context(ExitStack())
    gpool = pre1.enter_context(tc.tile_pool(name="gpool", bufs=1))
    e_n = {}
    for side, xh in (("1", io["x1"]), ("2", io["x2"])):
        for b in range(BL):
            idx = gpool.tile([128, 1], I32, tag=f"idx{side}_{b}", name=f"idx{side}_{b}")
            nc.sync.dma_start(out=idx[:, :], in_=xh[b, :])
            e = gpool.tile([128, D], H16, tag=f"e{side}_{b}", name=f"e{side}_{b}")
            nc.gpsimd.indirect_dma_start(
                out=e[:, :], out_offset=None, in_=io["emb"][:, :],
                in_offset=bass.IndirectOffsetOnAxis(ap=idx[:, :1], axis=0),
            )
            e_n[(side, b)] = e

    W = {}
    for n in WEIGHT_NAMES:
        W[n] = load_b(n) if n.endswith(("bh", "bt", "b1", "b2", "_b")) else load_w(n)

    # ---------------- helpers ----------------
    def mm_apply(w_tiles, b_tiles, rhs_tiles, n_free, func, out_tiles,
                 krange=None, mrange=None):
        """out = func(W.T @ rhs + b), transposed layout, 512-col PSUM chunks."""
        M = w_tiles[0].shape[1]
        mch = _chunks(M)
        ks = list(range(len(w_tiles))) if krange is None else krange
        m_iter = ([(i, i) for i in range(len(mch))] if mrange is None
                  else list(enumerate(mrange)))
        for oi, mi in m_iter:
            mo, mc = mch[mi]
            for fo in range(0, n_free, 512):
                fc = min(512, n_free - fo)
                ps = pp_mm.tile([128, 512], F32, tag="mmout", name="mmout")
                for idx, ki in enumerate(ks):
                    kc = w_tiles[ki].shape[0]
                    nc.tensor.matmul(
                        out=ps[:mc, :fc],
                        lhsT=w_tiles[ki][:kc, mo:mo + mc],
                        rhs=rhs_tiles[ki][:kc, fo:fo + fc],
                        start=(idx == 0),
                        stop=(idx == len(ks) - 1),
                    )
                nc.scalar.activation(
                    out=out_tiles[oi][:mc, fo:fo + fc],
                    in_=ps[:mc, :fc],
                    func=func, bias=b_tiles[mi][:mc, :], scale=1.0,
                )

    def highway(xt_tiles, wh, bh, wt, bt, feat, out_tiles):
        """out = x + t*(h-x), trunk layout, chunk-at-a-time (h reused as tmp)."""
        ch = _chunks(feat)
        for mi, (mo, mc) in enumerate(ch):
            h = work.tile([128, ROWS2], TRUNK, tag="hw_h", name="hw_h")
            t = work.tile([128, ROWS2], TRUNK, tag="hw_t", name="hw_t")
            mm_apply(wh, bh, xt_tiles, ROWS2, ACTF.Relu, [h], mrange=[mi])
            mm_apply(wt, bt, xt_tiles, ROWS2, ACTF.Sigmoid, [t], mrange=[mi])
            x_sl = xt_tiles[mi][:mc, :]
            nc.vector.tensor_tensor(out=h[:mc, :], in0=h[:mc, :], in1=x_sl,
                                    op=ALU.subtract)
            nc.vector.tensor_tensor(out=h[:mc, :], in0=h[:mc, :], in1=t[:mc, :],
                                    op=ALU.mult)
            nc.vector.tensor_tensor(out=out_tiles[mi][:mc, :], in0=h[:mc, :],
                                    in1=x_sl, op=ALU.add)

    # ---------------- embed: transpose into trunk ----------------
    # eT[ki]: [kc, 1024], col = side*512 + b*128 + token
    eT = [g2pool.tile([128, ROWS2], TRUNK, tag=f"eT_{i}", name=f"eT_{i}")
          for i in range(3)]
    for ki, (ko, kc) in enumerate(CH_D):
        for side in ("1", "2"):
            ps = pp_tr.tile([128, 512], H16, tag="trpackb", name="trpack")
            for b in range(BL):
                nc.tensor.transpose(
                    out=ps[:kc, b * S:(b + 1) * S],
                    in_=e_n[(side, b)][:, ko:ko + kc],
                    identity=identb[:128, :128],
                )
            so = (0 if side == "1" else ROWS)
            nc.scalar.activation(out=eT[ki][:kc, so:so + ROWS], in_=ps[:kc, :ROWS],
                                 func=ACTF.Copy)
    pre1.close()  # frees index + gather tiles

    # ---------------- highway stack (trunk: both sides at once) -------------
    h1 = [g2pool.tile([128, ROWS2], TRUNK, tag=f"hwy1_{i}", name=f"hwy1_{i}")
          for i in range(3)]
    highway(eT, W["hw1_Wh"], W["hw1_bh"], W["hw1_Wt"], W["hw1_bt"], D, h1)
    eTh = [persist.tile([128, ROWS2], TRUNK, tag=f"eTh_{i}", name=f"eTh_{i}")
           for i in range(3)]
    highway(h1, W["hw2_Wh"], W["hw2_bh"], W["hw2_Wt"], W["hw2_bt"], D, eTh)
    pre2.close()  # frees eT, h1

    # normal-layout post-highway embeddings (lhsT for the beta/alpha matmuls)
    ehw_n = {}
    for side in ("1", "2"):
        so = (0 if side == "1" else ROWS)
        for b in range(BL):
            ps = pp_r.tile([128, 512], TRUNK, tag="trpackr", name="trpackr")
            for ki, (ko, kc) in enumerate(CH_D):
                nc.tensor.transpose(
                    out=ps[:128, ko:ko + kc],
                    in_=eTh[ki][:kc, so + b * S:so + (b + 1) * S],
                    identity=identr[:kc, :kc],
                )
            t = persist.tile([128, D], H16, tag=f"ehwn{side}_{b}", name=f"ehwn{side}_{b}")
            nc.scalar.activation(out=t[:, :], in_=ps[:, :D], func=ACTF.Copy)
            ehw_n[(side, b)] = t

    # ---------------- projections (shared weights, trunk) ----------------
    def proj(prefix, pool):
        z1 = [work.tile([128, ROWS2], TRUNK, tag=f"z1_{i}", name=f"z1_{i}") for i in range(2)]
        mm_apply(W[f"{prefix}_W1"], W[f"{prefix}_b1"], eTh, ROWS2, ACTF.Relu, z1)
        out = [pool.tile([128, ROWS2], TRUNK, tag=f"{prefix}T_{i}", name=f"{prefix}T_{i}")
               for i in range(2)]
        mm_apply(W[f"{prefix}_W2"], W[f"{prefix}_b2"], z1, ROWS2, ACTF.Relu, out)
        return out

    # dist first so the att2 elementwise can start while the PE continues
    # with the mul projection (att1 is only needed at b0's first fold_sum)
    qscope = ctx.enter_context(ExitStack())
    qpool = qscope.enter_context(tc.tile_pool(name="qpool", bufs=1))
    qT = proj("dist", qpool)
    dbg_dump("dbg_eTh0", eTh[0][:, :])
    dbg_dump("dbg_qT0", qT[0][:, :])

    # fp16 views of q for the att2 elementwise: q1p = q1+1, q2b = q2.
    # The unused partition rows 72:128 of the low chunk are zeroed so the
    # subtract can run over all 128 partitions without stale data.
    q1p, q2b = [], []
    for ki in range(2):
        kc = CH_P[ki][1]
        tp = persist.tile([128, ROWS], H16, tag=f"q1p_{ki}", name=f"q1p_{ki}")
        t2 = persist.tile([128, ROWS], H16, tag=f"q2b_{ki}", name=f"q2b_{ki}")
        if kc < 128:
            nc.vector.memset(tp[64:128, :], 0.0)
            nc.vector.memset(t2[64:128, :], 0.0)
        nc.vector.tensor_scalar_add(out=tp[:kc, :], in0=qT[ki][:kc, :ROWS], scalar1=1.0)
        nc.vector.tensor_scalar_add(out=t2[:kc, :], in0=qT[ki][:kc, ROWS:], scalar1=0.0)
        q1p.append(tp)
        q2b.append(t2)
    qscope.close()  # frees qT

    pT = proj("mul", persist)

    # ---------------- att1 into sim4 PSUM (simT layout [j, i] per b) --------
    # start=True lazily marks the WHOLE bank pending-zero, so only the very
    # first matmul touching sim4 carries it; every later first-touch of a byte
    # overwrites, and overlapping writes accumulate. The att2 sums then
    # accumulate on top with start=False.
    sim4 = pp_sim.tile([128, 512], F32, tag="sim4", name="sim4")
    for b in range(BL):
        for ki, (ko, kc) in enumerate(CH_P):
            nc.tensor.matmul(
                out=sim4[:, b * S:(b + 1) * S],
                lhsT=pT[ki][:kc, ROWS + b * S:ROWS + (b + 1) * S],
                rhs=pT[ki][:kc, b * S:(b + 1) * S],
                start=(b == 0 and ki == 0), stop=False, skip_group_check=True,
            )

    # ---------------- att2: dist attention ----------------
    # Each j-block covers ALL 4 batch items: u layout [p, (b, j, i)]. The
    # subtracts run as small per-(b, chunk) instructions (good pipelining);
    # the rest of the elementwise runs whole-tile; the partition-sum matmuls
    # take a b-strided 512-wide rhs per j and write all of sim4's rows.
    half = JB * 512  # hi/lo chunk size in u (b, j, i)

    def att2_block(jb, type_a):
        u = upool.tile([128, 2 * half], H16, tag="u", name="u")
        for ki, off in ((0, 0), (1, half)):
            for b in range(BL):
                uo = u[:128, off + b * JB * S:off + (b + 1) * JB * S].rearrange(
                    "p (j i) -> p j i", j=JB)
                q1s = q1p[ki][:128, b * S:(b + 1) * S]
                in0 = bass.AP(tensor=q1s.tensor, offset=q1s.offset,
                              ap=[q1s.ap[0], [0, JB], q1s.ap[1]])
                q2s = q2b[ki][:128, b * S + jb * JB:b * S + (jb + 1) * JB]
                in1 = bass.AP(tensor=q2s.tensor, offset=q2s.offset,
                              ap=[q2s.ap[0], q2s.ap[1], [0, S]])
                nc.vector.tensor_tensor(out=uo, in0=in0, in1=in1,
                                        op=ALU.subtract)
        if type_a:
            # v = 2-u = 1-x ; u = max(u,v) = 1+|x| ; u = 1/u
            v = vpool.tile([128, 2 * half], H16, tag="v", name="v")
            nc.vector.tensor_scalar(out=v[:, :], in0=u[:, :], scalar1=-1.0,
                                    scalar2=2.0, op0=ALU.mult, op1=ALU.add)
            nc.vector.tensor_tensor(out=u[:, :], in0=u[:, :], in1=v[:, :],
                                    op=ALU.max)
            act_recip(nc, u[:, :], u[:, :], bias=0.0)
        else:
            # u = |u-1| = |x| ; u = 1/(1+u)
            nc.scalar.activation(out=u[:, :], in_=u[:, :], func=ACTF.Abs,
                                 bias=neg1[:, :])
            act_recip(nc, u[:, :], u[:, :], bias=1.0)
        return u

    def att2_fold_sum(jb, u):
        # fold the p=128..200 chunk onto the first 72 rows of the hi chunk
        # (DVE only: gpsimd takes ~27us per fold and gates the pipeline)
        nc.vector.tensor_tensor(out=u[:72, :half], in0=u[:72, :half],
                                in1=u[:72, half:], op=ALU.add)
        for jj in range(JB):
            j = jb * JB + jj
            g, rr = j // 32, j % 32
            rbase = u[:128, jj * S:jj * S + S]
            rhs = bass.AP(tensor=rbase.tensor, offset=rbase.offset,
                          ap=[rbase.ap[0], [JB * S, BL], [1, S]])
            nc.tensor.matmul(
                out=sim4[32 * g:32 * g + 32, :],
                lhsT=zbuf[:128, 32 - rr:64 - rr],
                rhs=rhs,
                start=False, stop=(rr == 31), skip_group_check=True,
                tile_position=(0, 32 * g),
            )

    prev = None
    for jb in range(NBLK):
        type_a = (jb * A_NUM) % A_DEN < A_NUM
        u = att2_block(jb, type_a)
        if prev is not None:
            att2_fold_sum(*prev)
        prev = (jb, u)
    att2_fold_sum(*prev)
    dbg_dump("dbg_sim4", sim4[:, :])

    # ---------------- softmax + beta/alpha + compare part 1 ----------------
    def softmax_p(src_psum):
        """softmax over rows of src [128,128]; returns transposed probs bf16."""
        mx = small.tile([128, 1], F32, tag="sm_mx", name="sm_mx")
        nc.vector.tensor_reduce(out=mx[:, :], in_=src_psum, axis=AX.X,
                                op=ALU.max, negate=True)
        esb = small.tile([128, S], H16, tag="sm_e", name="sm_e")
        zs = small.tile([128, 1], F32, tag="sm_z", name="sm_z")
        nc.scalar.activation(out=esb[:, :], in_=src_psum, func=ACTF.Exp,
                             bias=mx[:, :], scale=1.0, accum_out=zs[:, :])
        rz = small.tile([128, 1], F32, tag="sm_rz", name="sm_rz")
        nc.vector.reciprocal(out=rz[:, :], in_=zs[:, :])
        pr = small.tile([128, S], H16, tag="sm_p", name="sm_p")
        nc.vector.tensor_scalar(out=pr[:, :], in0=esb[:, :], scalar1=rz[:, :],
                                scalar2=None, op0=ALU.mult)
        pt_ps = pp_tr.tile([128, 512], H16, tag="trpackb", name="trpackb")
        nc.tensor.transpose(out=pt_ps[:S, :S], in_=pr[:, :], identity=identb[:, :])
        pt = small.tile([128, S], H16, tag="sm_pt", name="sm_pt")
        nc.scalar.activation(out=pt[:, :], in_=pt_ps[:S, :S], func=ACTF.Copy)
        return pt

    # betaT trunk tiles per side: [kc, 512] bf16, col = b*128 + token
    betaT = {s: [persist.tile([128, 512], H16, tag=f"betaT{s}_{i}", name=f"betaT{s}_{i}")
                 for i in range(3)] for s in ("1", "2")}

    for b in range(BL):
        bs4 = sim4[:, b * S:(b + 1) * S]
        ptA = softmax_p(bs4)  # alpha probs^T [i, j]
        simT_sb = small.tile([128, S], F32, tag="simT_sb", name="simT_sb")
        nc.scalar.activation(out=simT_sb[:, :], in_=bs4, func=ACTF.Copy)
        sim_ps = pp_sm.tile([128, S], F32, tag="btps", name="simtr")
        nc.tensor.transpose(out=sim_ps[:S, :S], in_=simT_sb[:, :],
                            identity=identf[:, :])
        ptB = softmax_p(sim_ps[:S, :S])  # beta probs^T [j, i]

        for side, pt, eln in (("1", ptB, "2"), ("2", ptA, "1")):
            for ki, (ko, kc) in enumerate(CH_D):
                bt_ps = pp_sm.tile([128, S], F32, tag="btps", name="btps")
                nc.tensor.matmul(
                    out=bt_ps[:kc, :], lhsT=ehw_n[(eln, b)][:, ko:ko + kc],
                    rhs=pt[:, :], start=True, stop=True,
                )
                nc.scalar.activation(
                    out=betaT[side][ki][:kc, b * S:(b + 1) * S],
                    in_=bt_ps[:kc, :], func=ACTF.Copy)

    dbg_dump("dbg_betaT0", betaT["1"][0][:, :])

    # cat + compare matmul, per side over 512-col trunk halves. The cat
    # chunks (e-b, e*b) are computed on the fly right before their two
    # accumulating matmuls, so only 2 transient tiles are alive at a time.
    cmp1 = [persist.tile([128, ROWS2], H16, tag=f"cmp1_{i}", name=f"cmp1_{i}")
            for i in range(2)]
    for side in ("1", "2"):
        so = (0 if side == "1" else ROWS)
        ps2 = [pp_mm.tile([128, 512], F32, tag="mmout", name=f"cmp1ps{mi}")
               for mi in range(2)]
        for sel in range(4):  # e, beta, e-beta, e*beta
            for ki, (ko, kc) in enumerate(CH_D):
                e_sl = eTh[ki][:kc, so:so + ROWS]
                b_sl = betaT[side][ki][:kc, :]
                if sel == 1:
                    rhs = b_sl
                else:
                    cat = small.tile([128, 512], H16, tag="cat", name="cat")
                    if sel == 0:
                        nc.vector.tensor_scalar_add(out=cat[:kc, :], in0=e_sl,
                                                    scalar1=0.0)
                    else:
                        nc.vector.tensor_tensor(
                            out=cat[:kc, :], in0=e_sl, in1=b_sl,
                            op=(ALU.subtract if sel == 2 else ALU.mult))
                    rhs = cat[:kc, :]
                idx = sel * 3 + ki
                for mi, (mo, mc) in enumerate(CH_P):
                    nc.tensor.matmul(
                        out=ps2[mi][:mc, :],
                        lhsT=W["cmp_W1"][idx][:, mo:mo + mc],
                        rhs=rhs,
                        start=(idx == 0), stop=(idx == 11),
                        skip_group_check=True,
                    )
        for mi, (mo, mc) in enumerate(CH_P):
            nc.scalar.activation(
                out=cmp1[mi][:mc, so:so + ROWS], in_=ps2[mi][:mc, :],
                func=ACTF.Relu,
                bias=W["cmp_b1"][mi][:mc, :], scale=1.0,
            )

    # ---------------- compare part 2 + compare highway (trunk) --------------
    v0 = [work.tile([128, ROWS2], H16, tag=f"v0_{i}", name=f"v0_{i}") for i in range(2)]
    mm_apply(W["cmp_W2"], W["cmp_b2"], cmp1, ROWS2, ACTF.Relu, v0)
    v1 = [work.tile([128, ROWS2], H16, tag=f"v1_{i}", name=f"v1_{i}") for i in range(2)]
    highway(v0, W["chw1_Wh"], W["chw1_bh"], W["chw1_Wt"], W["chw1_bt"], P, v1)
    vT = [persist.tile([128, ROWS2], H16, tag=f"vT_{i}", name=f"vT_{i}")
          for i in range(2)]
    highway(v1, W["chw2_Wh"], W["chw2_bh"], W["chw2_Wt"], W["chw2_bt"], P, vT)
    dbg_dump("dbg_vT0", vT[0][:, :])

    # ---------------- aggregate (fp32) ----------------
    # stats[sect][ki]: [kc, BL]; sections: v1.max, v2.max, v1.sum, v2.sum
    stats = []
    for sect, (side, op) in enumerate(
            (("1", ALU.max), ("2", ALU.max), ("1", ALU.add), ("2", ALU.add))):
        so = (0 if side == "1" else ROWS)
        st = [persist.tile([128, BL], F32, tag=f"st{sect}_{i}", name=f"st{sect}_{i}")
              for i in range(2)]
        for ki, (ko, kc) in enumerate(CH_P):
            seg = vT[ki][:kc, so:so + ROWS].rearrange("p (b t) -> p b t", b=BL)
            nc.vector.tensor_reduce(
                out=st[ki][:kc, :BL], in_=seg, axis=AX.X, op=op,
            )
        stats.append(st)

    agg_rhs = [stats[s][ki] for s in range(4) for ki in range(2)]
    y1 = [persist.tile([128, BL], F32, tag=f"y1_{i}", name=f"y1_{i}") for i in range(2)]
    mm_apply(W["agg_W1"], W["agg_b1"], agg_rhs, BL, ACTF.Relu, y1)
    y2 = [persist.tile([128, BL], F32, tag=f"y2_{i}", name=f"y2_{i}") for i in range(2)]
    mm_apply(W["agg_W2"], W["agg_b2"], y1, BL, ACTF.Relu, y2)

    yt_ps = pp_sm.tile([128, S], F32, tag="btps", name="btps")
    for ki, (ko, kc) in enumerate(CH_P):
        nc.tensor.matmul(
            out=yt_ps[:C, :BL], lhsT=W["out_W"][ki][:kc, :],
            rhs=y2[ki][:kc, :], start=(ki == 0), stop=(ki == 1),
        )
    yt_sb = persist.tile([C, BL], F32, tag="yt_sb", name="yt_sb")
    nc.scalar.activation(out=yt_sb[:, :], in_=yt_ps[:C, :BL], func=ACTF.Identity,
                         bias=W["out_b"][0][:C, :], scale=1.0)
    nc.sync.dma_start(out=io["yt"][:, :], in_=yt_sb[:, :])


_NC_CACHE = {}


def _get_nc():
    if "nc" not in _NC_CACHE:
        _NC_CACHE["nc"] = build_nc()
    return _NC_CACHE["nc"]


def make_in_maps(inputs):
    """Shard full inputs into 8 per-core input maps."""
    x1 = np.ascontiguousarray(np.asarray(inputs["x1"]).astype(np.int32))
    x2 = np.ascontiguousarray(np.asarray(inputs["x2"]).astype(np.int32))
    shared = {}
    for n in WEIGHT_NAMES + ["emb"]:
        shared[n] = np.ascontiguousarray(np.asarray(inputs[n]).astype(np.float32))
    in_maps = []
    for c in range(NCORES):
        m = dict(shared)
        m["x1"] = x1[c * BL:(c + 1) * BL]
        m["x2"] = x2[c * BL:(c + 1) * BL]
        in_maps.append(m)
    return in_maps


def kernel(**inputs):
    nc = _get_nc()
    in_maps = make_in_maps(inputs)
    res = run_bass_kernel_spmd(nc, in_maps, core_ids=list(range(NCORES)))
    return np.concatenate([np.asarray(r["yt"]).T for r in res.results], axis=0)


if __name__ == "__main__":
    nc = build_nc()
    print("built ok")
